# revision 111
# baseline (speedup 1.0000x reference)
"""Trainium2 Bass kernel for BasicTransformerBlockST (spatial/temporal block).

Sharding over 8 NeuronCores (same as baseline):
  Phase A (spatial self-attn): data-parallel over (b,t): core i owns the 4
  groups bt = i + 8g. An 8-way AllToAll (split in two, overlapped with phase
  A compute) reshards to (b,h,w)-parallel: core j owns rows
  (b=j//4, hw in [144*(j%4), 144*(j%4+1))), tokens r-major (token = r*16+t).
  t1 / cross-attn / t2 / FFN run on that shard with the residual stream
  resident in SBUF (no DRAM bounces).

Optimized for the TimelineSim cost model: batched big instructions, S^T
softmax formulation (no attention-matrix transposes or renormalize in phase
A / cross), z via ones-column fused into AV, evictions spread across
DVE/Act/Pool, PSUM tag sharing for double buffering.

Scheduling notes (engines execute their queues strictly in order, so
emission order is the schedule):
 - residual crosses the AllToAll in bf16; each slot is split into row-half
   collectives in separate DRAM tensors so t1 half-0 starts during the
   second collective
 - temporal/cross window loops are staged pipelines emitted in per-engine
   ready order (AB(k+1); CD(k); fa(k+2); back(k))
 - V projection is emitted after the window-loop prologue so its PE work
   fills the softmax pipeline-fill bubble
 - LN stats are batched per phase (one Sqrt act-table episode each)
 - a single matmul's PSUM output span must stay inside one 2KB bank
   (the o2 rel-V matmuls are split per head-half for this); accumulating
   with start=False onto a region written by a different matmul shape
   produces wrong results on this stack - keep o1/o2 in separate psum
"""

import sys

sys.path.insert(0, "/opt/trn_rl_repo")

import numpy as np
import ml_dtypes

import concourse.bass as bass
import concourse.bacc as bacc
import concourse.mybir as mybir
import concourse.tile as tile
from concourse.masks import make_identity

F32 = mybir.dt.float32
BF16 = mybir.dt.bfloat16
AF = mybir.ActivationFunctionType
ALU = mybir.AluOpType
AX = mybir.AxisListType

B, C, T, H, W = 2, 640, 16, 24, 24
HEADS, DH = 8, 80
CTXD = 1024
MAXREL = 16
NREL = 2 * MAXREL + 1          # 33
FFI = 4 * C                    # 2560
INNER = HEADS * DH             # 640
SCALE = DH ** -0.5
EPS = 1e-5

NCORES = 8
NG = 4                         # spatial groups per core
SEQ = H * W                    # 576
NR = (B * H * W) // NCORES     # 144 rows per core
TOK = NR * T                   # 2304 tokens per core
NWIN = TOK // 128              # 18
CHUNKS = C // 128              # 5
CTXCH = CTXD // 128            # 8
HALFW = NWIN // 2              # 9 windows per temporal half
HR = NR // 2                   # 72 rows per half
HTOK = 128 * HALFW             # 1152 tokens per half
NG2 = FFI // 128               # 20 ffn chunks

# token chunks of a 576-token spatial group
QSP = [(0, 128), (128, 128), (256, 128), (384, 128), (512, 64)]


PHASE_MARKS = []


def build_program(debug=False):
    nc = bacc.Bacc(None, target_bir_lowering=False)

    # instrument PE-instruction counts at phase boundaries (analysis only)
    PHASE_MARKS.clear()
    _mmcnt = [0]
    _omm = bass.BassTensorEngine.matmul

    def _cmm(self, *a, **k):
        _mmcnt[0] += 1
        return _omm(self, *a, **k)

    bass.BassTensorEngine.matmul = _cmm

    def _mark(name):
        PHASE_MARKS.append((name, _mmcnt[0]))

    xs_in = nc.dram_tensor("xs_in", [NG, SEQ, C], BF16, kind="ExternalInput")
    ctxT_in = nc.dram_tensor("ctxT", [CTXD, 77], BF16, kind="ExternalInput")

    def win(name, shape, dt=BF16):
        return nc.dram_tensor(name, shape, dt, kind="ExternalInput")

    wts = {}
    for p in ("a1", "a2", "t1", "t2"):
        cin = CTXD if p == "a2" else C
        wts[f"{p}_wq"] = win(f"{p}_wq", [C, INNER])
        wts[f"{p}_wk"] = win(f"{p}_wk", [cin, INNER])
        wts[f"{p}_wv"] = win(f"{p}_wv", [cin, INNER])
        wts[f"{p}_wo"] = win(f"{p}_wo", [DH, HEADS, C])
    for p in ("t1", "t2"):
        wts[f"{p}_rkT"] = win(f"{p}_rkT", [DH, NREL])
        wts[f"{p}_rvs"] = win(f"{p}_rvs", [16, T, DH])  # rvs[j,t,d]=rv[j-t+16,d]
    wts["ff_w1"] = win("ff_w1", [C, 2 * FFI])  # host-permuted cols (4a,4g)
    wts["ff_w2"] = win("ff_w2", [FFI, C])
    bd_mask = win("bd_mask", [128, 128], BF16)

    out_final = nc.dram_tensor("out", [NR, T, C], F32, kind="ExternalOutput")
    dbg = {}
    if debug:
        dbg["a"] = nc.dram_tensor("dbg_a", [NG, SEQ, C], BF16,
                                  kind="ExternalOutput")
        for nm in ("t1", "x2", "t2"):
            dbg[nm] = nc.dram_tensor(f"dbg_{nm}", [NR, T, C], F32,
                                     kind="ExternalOutput")
        dbg["aG"] = nc.dram_tensor("dbg_aG", [128, HEADS, 128], BF16,
                                   kind="ExternalOutput")
        dbg["v0"] = nc.dram_tensor("dbg_v0", [128, C], BF16,
                                   kind="ExternalOutput")
        dbg["q0"] = nc.dram_tensor("dbg_q0", [DH, HEADS, 128], BF16,
                                   kind="ExternalOutput")
        dbg["oT0"] = nc.dram_tensor("dbg_oT0", [DH, HEADS, 128], BF16,
                                    kind="ExternalOutput")

    # slot-major a2a: slot s holds frames t = i + 8*s from src core i.
    # One tensor per row half (rh) so the temporal phase's half-0 fill
    # only depends on the rh=0 collectives; slot is the leading dim.
    a2a_in = [nc.dram_tensor(f"a2a_in{r}", [2, NCORES, HR, C], BF16)
              for r in range(2)]
    a2a_out = [nc.dram_tensor(f"a2a_out{r}", [2, NCORES, HR, C], BF16)
               for r in range(2)]
    s2_dram = nc.dram_tensor("s2_dram", [TOK, HEADS, 16], BF16)
    groups = [[0, 1, 2, 3, 4, 5, 6, 7]]

    from contextlib import ExitStack

    with tile.TileContext(nc) as tc, ExitStack() as top:
        const = top.enter_context(tc.tile_pool(name="const", bufs=1))
        identb = const.tile([128, 128], BF16)
        make_identity(nc, identb)
        eps_t = const.tile([128, 1], F32)
        nc.vector.memset(eps_t[:], EPS)
        mask = const.tile([128, 128], BF16)
        nc.sync.dma_start(out=mask[:], in_=bd_mask[:, :])
        small = top.enter_context(tc.tile_pool(name="small", bufs=6))
        resp = top.enter_context(tc.tile_pool(name="resp", bufs=1))
        x_sb = resp.tile([128, NWIN, C], F32, tag="x_sb")

        ev_state = [0]

        def evict(out, in_, w=(1, 1, 1)):
            """psum->sbuf copy via rotating engines; w=(dve, act, act2).
            GPSIMD cannot access PSUM, so only DVE/Act here."""
            seq = [nc.vector] * w[0] + [nc.scalar] * (w[1] + w[2])
            eng = seq[ev_state[0] % len(seq)]
            ev_state[0] += 1
            if eng is nc.scalar:
                eng.copy(out=out, in_=in_)
            else:
                eng.tensor_copy(out=out, in_=in_)

        def ln_stats(sp, xfn, nw, tag):
            """Batched LN stats: one Sqrt activation for all nw windows so
            the Act engine swaps function tables once per phase, not per
            window. Returns (mv [128,nw,2], rstd [128,nw])."""
            mv = sp.tile([128, nw, 2], F32, tag=tag + "_mv")
            for w in range(nw):
                x = xfn(w)
                st = small.tile([128, 2, 6], F32, tag="bnst")
                nc.vector.bn_stats(out=st[:, 0, :], in_=x[:, 0:512])
                nc.vector.bn_stats(out=st[:, 1, :], in_=x[:, 512:640])
                nc.vector.bn_aggr(out=mv[:, w, :], in_=st[:])
            rstd = sp.tile([128, nw], F32, tag=tag + "_rs")
            mva = mv[:, :, :]
            var = bass.AP(tensor=mv.tensor, offset=mva.offset + 1,
                          ap=[list(mva.ap[0]), [2, nw]])
            nc.scalar.activation(out=rstd[:], in_=var, func=AF.Sqrt,
                                 bias=eps_t[:], scale=1.0)
            nc.vector.reciprocal(out=rstd[:], in_=rstd[:])
            return mv, rstd

        def ln_fm(psp, zp, xfn, zT, nw, stats=None, w0=0):
            """LayerNorm (g/b folded into weights) + transpose into
            feature-major zT[:, ci, 128*w : 128*w+128] bf16."""
            zT_a = zT[:, :, :]
            ntok = zT_a.ap[1][0]
            if stats is None:
                stats = ln_stats(zp, xfn, nw, "lnf")
            mv, rstd = stats
            for w in range(nw):
                x = xfn(w)
                zs = zp.tile([128, C], BF16, tag="zs")
                nc.vector.tensor_scalar(
                    out=zs[:], in0=x, scalar1=mv[:, w0 + w, 0:1],
                    scalar2=rstd[:, w0 + w:w0 + w + 1],
                    op0=ALU.subtract, op1=ALU.mult)
                pz = psp.tile([128, CHUNKS, 128], BF16, tag="pz")
                for c in range(CHUNKS):
                    nc.tensor.transpose(pz[:, c, :], zs[:, 128 * c:128 * (c + 1)],
                                        identb[:])
                dst = bass.AP(tensor=zT.tensor,
                              offset=zT_a.offset + 128 * w,
                              ap=[list(zT_a.ap[0]), [ntok, CHUNKS], [1, 128]])
                evict(dst, pz[:, :, :], w=(2, 1, 1))

        def load_w_cin(wp, name, cin):
            t = wp.tile([128, cin // 128, wts[name].shape[-1]], BF16,
                        tag="w_" + name)
            nc.sync.dma_start(out=t[:],
                              in_=wts[name][:].rearrange("(a p) n -> p a n", p=128))
            return t

        def load_wo(wp, name):
            t = wp.tile([DH, HEADS, C], BF16, tag="w_" + name)
            nc.sync.dma_start(out=t[:], in_=wts[name][:])
            return t

        def proj_fm(psp, zT, w_sb, qT, ntok):
            """feature-major projection qT[80, h, ntok] (bf16).
            PSUM allocations cap at 4KB, so one 1-bank tile per 512-split."""
            for h in range(HEADS):
                for o in range(0, ntok, 512):
                    n = min(512, ntok - o)
                    pq = psp.tile([128, 512], F32, tag="pA")
                    for ci in range(CHUNKS):
                        nc.tensor.matmul(pq[:DH, 0:n],
                                         w_sb[:, ci, DH * h:DH * (h + 1)],
                                         zT[:, ci, o:o + n],
                                         start=(ci == 0), stop=(ci == CHUNKS - 1))
                    evict(qT[:, h, o:o + n], pq[:DH, 0:n], w=(2, 2, 1))

        def wo_resid(psp, tag, oT, qoff, ntok, wo_sb, resid_ap):
            """WO projection (by-head lhsT oT[:, h, qoff:qoff+ntok]) +
            residual add into resid_ap [ntok, C]."""
            pw = psp.tile([128, 1024], F32, tag=tag)
            for o, n in ((0, 512), (512, 128)):
                for h in range(HEADS):
                    nc.tensor.matmul(pw[:ntok, o:o + n],
                                     oT[:, h, qoff:qoff + ntok],
                                     wo_sb[:, h, o:o + n],
                                     start=(h == 0), stop=(h == HEADS - 1))
            nc.vector.scalar_tensor_tensor(
                out=resid_ap, in0=pw[:ntok, 0:C], scalar=1.0, in1=resid_ap,
                op0=ALU.mult, op1=ALU.add)

        # =====================================================================
        # PHASE A: spatial self-attention per (b,t) group; order 0,2,1,3 so
        # each a2a slot's collective fires after two groups.
        # =====================================================================
        with ExitStack() as ph:
            xap = ph.enter_context(tc.tile_pool(name="xapA", bufs=1))
            wp = ph.enter_context(tc.tile_pool(name="wpA", bufs=1))
            zp = ph.enter_context(tc.tile_pool(name="zpA", bufs=2))
            qp = ph.enter_context(tc.tile_pool(name="qpA", bufs=2))
            ap_ = ph.enter_context(tc.tile_pool(name="apA", bufs=2))
            psp = ph.enter_context(tc.tile_pool(name="psA", bufs=3, space="PSUM"))
            pso = ph.enter_context(tc.tile_pool(name="psoA", bufs=2, space="PSUM"))

            # all 4 groups' inputs DMA'd up front (before the weight loads in
            # the DMA queue) + one batched LN-stats pass: a single Sqrt table
            # episode for the whole phase instead of one per group
            xall = xap.tile([128, NG, CHUNKS, C], BF16, tag="xall")
            for g in (0, 2, 1, 3):
                nc.sync.dma_start(out=xall[:, g, 0:4, :],
                                  in_=xs_in[g, 0:512, :].rearrange(
                                      "(a p) c -> p a c", p=128))
                nc.sync.dma_start(out=xall[:64, g, 4, :],
                                  in_=xs_in[g, 512:576, :])
            statsA = ln_stats(wp, lambda k: xall[:, k // 5, k % 5, :],
                              NG * CHUNKS, "lnA")

            wq = load_w_cin(wp, "a1_wq", C)
            wk = load_w_cin(wp, "a1_wk", C)
            wv = load_w_cin(wp, "a1_wv", C)
            wo = load_wo(wp, "a1_wo")

            for g in (0, 2, 1, 3):
                zT = zp.tile([128, CHUNKS, 640], BF16, tag="zTa")
                ln_fm(pso, zp, lambda w, g=g: xall[:, g, w, :], zT, 5,
                      stats=statsA, w0=5 * g)

                qT = qp.tile([DH, HEADS, SEQ], BF16, tag="qa")
                kT = qp.tile([DH, HEADS, SEQ], BF16, tag="ka")
                proj_fm(psp, zT, wq, qT, SEQ)
                proj_fm(psp, zT, wk, kT, SEQ)

                # v token-major with ones column per head (memset 1.0 first;
                # the projection evictions overwrite all but the ones column)
                v1 = qp.tile([128, CHUNKS, HEADS, DH + 1], BF16, tag="va")
                nc.gpsimd.memset(v1[:], 1.0)
                for (w, (o_, np_)) in enumerate(QSP):
                    pv = psp.tile([128, 1024], F32, tag="pA")
                    for o, n in ((0, 512), (512, 128)):
                        for ci in range(CHUNKS):
                            nc.tensor.matmul(pv[:np_, o:o + n],
                                             zT[:, ci, o_:o_ + np_],
                                             wv[:, ci, o:o + n],
                                             start=(ci == 0), stop=(ci == CHUNKS - 1))
                    v1a = v1[:, :, :, :]
                    dst = bass.AP(tensor=v1.tensor,
                                  offset=v1a.offset + w * HEADS * (DH + 1),
                                  ap=[[v1a.ap[0][0], np_], [DH + 1, HEADS],
                                      [1, DH]])
                    evict(dst, pv[:np_, 0:C], w=(2, 1, 1))

                oT = ap_.tile([DH, HEADS, SEQ], BF16, tag="oa")

                def a_front(h):
                    """scores exp(S^T) for head h"""
                    eS = ap_.tile([128, CHUNKS, SEQ], BF16, tag="eS")
                    for (kc, (ko, kp)) in enumerate(QSP):
                        ps = psp.tile([128, 1024], F32, tag="pA")
                        for o, n in ((0, 512), (512, 64)):
                            nc.tensor.matmul(ps[:kp, o:o + n],
                                             kT[:, h, ko:ko + kp],
                                             qT[:, h, o:o + n],
                                             start=True, stop=True)
                        nc.scalar.activation(out=eS[:kp, kc, 0:SEQ],
                                             in_=ps[:kp, 0:SEQ],
                                             func=AF.Exp, scale=SCALE)
                    return eS

                def a_back(h, eS):
                    # AV + z via ones column: oA[q, 80] = z
                    oA = pso.tile([128, CHUNKS, 96], F32, tag="pz")
                    for (qc, (qo, qp_)) in enumerate(QSP):
                        for (kc, (ko, kp)) in enumerate(QSP):
                            nc.tensor.matmul(oA[:qp_, qc, 0:DH + 1],
                                             eS[:kp, kc, qo:qo + qp_],
                                             v1[:kp, kc, h, :],
                                             start=(kc == 0), stop=(kc == 4))
                    rz = small.tile([128, CHUNKS], F32, tag="rz")
                    oAa = oA[:, :, :]
                    zv = bass.AP(tensor=oA.tensor, offset=oAa.offset + DH,
                                 ap=[list(oAa.ap[0]), [96, CHUNKS]])
                    nc.vector.reciprocal(out=rz[:], in_=zv)
                    oN = ap_.tile([128, CHUNKS, DH], BF16, tag="oN")
                    src = bass.AP(tensor=oA.tensor, offset=oAa.offset,
                                  ap=[list(oAa.ap[0]), [96, CHUNKS], [1, DH]])
                    rza = rz[:, :]
                    rzb = bass.AP(tensor=rz.tensor, offset=rza.offset,
                                  ap=[list(rza.ap[0]), [1, CHUNKS], [0, DH]])
                    nc.vector.tensor_tensor(out=oN[:], in0=src, in1=rzb,
                                            op=ALU.mult)
                    pt = pso.tile([DH, CHUNKS, 128], BF16, tag="pz")
                    for (qc, (qo, qp_)) in enumerate(QSP):
                        nc.tensor.transpose(pt[:, qc, 0:qp_], oN[:qp_, qc, :],
                                            identb[:qp_, :qp_])
                    pta = pt[:, :, :]
                    src = bass.AP(tensor=pt.tensor, offset=pta.offset,
                                  ap=[list(pta.ap[0]), [128, 4], [1, 128]])
                    evict(oT[:, h, 0:512], src, w=(2, 1, 1))
                    evict(oT[:, h, 512:576], pt[:, 4, 0:64], w=(2, 1, 1))

                # software-pipeline heads: scores(h+1) before AV/norm(h) so
                # the PE never waits on head h's exp chain
                prev_eS = None
                for h in range(HEADS):
                    eS = a_front(h)
                    if prev_eS is not None:
                        a_back(h - 1, prev_eS)
                    prev_eS = eS
                a_back(HEADS - 1, prev_eS)

                # WO + residual (in place on xg), cast to bf16 for the
                # collective, scatter to a2a_in
                b_, tslot = g // 2, g % 2
                for (qc, (qo, qp_)) in enumerate(QSP):
                    xq = xall[:qp_, g, qc, :]
                    wo_resid(psp, "pA", oT, qo, qp_, wo, xq)
                    q0, q1 = qo // HR, (qo + qp_ - 1) // HR
                    for q in range(q0, q1 + 1):
                        lo, hi = max(qo, HR * q), min(qo + qp_, HR * (q + 1))
                        nc.sync.dma_start(
                            out=a2a_in[q % 2][tslot, 4 * b_ + q // 2,
                                              lo - HR * q:hi - HR * q, :],
                            in_=xall[lo - qo:hi - qo, g, qc, :])
                    if debug:
                        nc.sync.dma_start(out=dbg["a"][g, qo:qo + qp_, :],
                                          in_=xall[:qp_, g, qc, :])
                if g == 2:
                    for r in range(2):
                        nc.gpsimd.collective_compute(
                            "AllToAll", ALU.bypass, replica_groups=groups,
                            ins=[a2a_in[r][0]], outs=[a2a_out[r][0]])
            for r in range(2):
                nc.gpsimd.collective_compute(
                    "AllToAll", ALU.bypass, replica_groups=groups,
                    ins=[a2a_in[r][1]], outs=[a2a_out[r][1]])

        # cross-attention KV setup hoisted here: it has no dependency on
        # the AllToAll, so PE/DMA work lands inside the collective gap
        xkv = top.enter_context(tc.tile_pool(name="xkv", bufs=1))
        wqx = load_w_cin(xkv, "a2_wq", C)
        wox = load_wo(xkv, "a2_wo")
        with ExitStack() as hs:
            kvp = hs.enter_context(tc.tile_pool(name="kvpX", bufs=1))
            psk = hs.enter_context(tc.tile_pool(name="pskX", bufs=2,
                                                space="PSUM"))
            wkc = load_w_cin(kvp, "a2_wk", CTXD)
            wvc = load_w_cin(kvp, "a2_wv", CTXD)
            ctx_sb = kvp.tile([128, CTXCH, 77], BF16, tag="ctx")
            nc.sync.dma_start(out=ctx_sb[:],
                              in_=ctxT_in[:].rearrange("(a p) m -> p a m",
                                                       p=128))
            kctxT = xkv.tile([DH, HEADS, 77], BF16, tag="kctx")
            pk = psk.tile([DH, HEADS, 128], F32, tag="pk")
            for h in range(HEADS):
                for ci in range(CTXCH):
                    nc.tensor.matmul(pk[:, h, 0:77],
                                     wkc[:, ci, DH * h:DH * (h + 1)],
                                     ctx_sb[:, ci, :],
                                     start=(ci == 0), stop=(ci == CTXCH - 1))
            pka = pk[:, :, :]
            src = bass.AP(tensor=pk.tensor, offset=pka.offset,
                          ap=[list(pka.ap[0]), [128, HEADS], [1, 77]])
            evict(kctxT[:, :, :], src, w=(1, 1, 1))
            v1x = xkv.tile([77, HEADS, DH + 1], BF16, tag="vctx")
            nc.gpsimd.memset(v1x[:], 1.0)
            pv = psk.tile([77, 1024], F32, tag="pvx")
            for o, n in ((0, 512), (512, 128)):
                for ci in range(CTXCH):
                    nc.tensor.matmul(pv[:, o:o + n], ctx_sb[:, ci, :],
                                     wvc[:, ci, o:o + n],
                                     start=(ci == 0), stop=(ci == CTXCH - 1))
            v1a = v1x[:, :, :]
            dst = bass.AP(tensor=v1x.tensor, offset=v1a.offset,
                          ap=[list(v1a.ap[0]), [DH + 1, HEADS], [1, DH]])
            evict(dst, pv[:, 0:C], w=(1, 1, 1))

        # fill x_sb windows from a2a_out: partition p=16r'+t, t=i+8s;
        # bf16 staging + per-window upcast back to the f32 residual.
        # Done per temporal half (scoped pool) so half-0's pipeline never
        # waits on the half-1 collective.
        def fill_half(half, pool):
            xst = pool.tile([128, HALFW, C], BF16, tag="xst")
            base = a2a_out[half][:]
            for i in range(HALFW):
                src = bass.AP(tensor=base.tensor,
                              offset=base.offset + 8 * i * C,
                              ap=[[C, 8], [NCORES * HR * C, 2],
                                  [HR * C, 8], [1, C]])
                nc.sync.dma_start(out=xst[:, i, :], in_=src)
                w = half * HALFW + i
                if i % 3 == 0:
                    nc.vector.tensor_copy(out=x_sb[:, w, :],
                                          in_=xst[:, i, :])
                elif i % 3 == 1:
                    nc.scalar.copy(out=x_sb[:, w, :], in_=xst[:, i, :])
                else:
                    nc.gpsimd.tensor_copy(out=x_sb[:, w, :],
                                          in_=xst[:, i, :])

        # =====================================================================
        # Temporal attention (t1 / t2), per half
        # =====================================================================
        def temporal(prefix, dbg_key, filler=None):
            with ExitStack() as ph:
                wp = ph.enter_context(tc.tile_pool(name="wpT", bufs=1))
                zp = ph.enter_context(tc.tile_pool(name="zpT", bufs=2))
                ztp = ph.enter_context(tc.tile_pool(name="ztpT", bufs=1))
                qp = ph.enter_context(tc.tile_pool(name="qpT", bufs=1))
                swp = ph.enter_context(tc.tile_pool(name="swpT", bufs=2))
                sp2 = ph.enter_context(tc.tile_pool(name="sp2T", bufs=2))

                wq = load_w_cin(wp, f"{prefix}_wq", C)
                wk = load_w_cin(wp, f"{prefix}_wk", C)
                wv = load_w_cin(wp, f"{prefix}_wv", C)
                wo = load_wo(wp, f"{prefix}_wo")
                rkT = wp.tile([DH, NREL], BF16, tag="rkT")
                nc.sync.dma_start(out=rkT[:], in_=wts[f"{prefix}_rkT"][:])
                rvs = wp.tile([16, T, DH], BF16, tag="rvs")
                nc.sync.dma_start(out=rvs[:], in_=wts[f"{prefix}_rvs"][:])

                for half in range(2):
                    if filler is not None:
                        filler(half, qp)
                    wlo = half * HALFW
                    zT = ztp.tile([128, CHUNKS, HTOK], BF16, tag="zTt")
                    with ExitStack() as hs:
                        psz = hs.enter_context(
                            tc.tile_pool(name="pszT", bufs=4, space="PSUM"))
                        ln_fm(psz, zp, lambda w: x_sb[:, wlo + w, :], zT, HALFW)
                    qT = qp.tile([DH, HEADS, HTOK], BF16, tag="qt")
                    kT = qp.tile([DH, HEADS, HTOK], BF16, tag="kt")
                    with ExitStack() as hs:
                        psq = hs.enter_context(
                            tc.tile_pool(name="psqT", bufs=6, space="PSUM"))
                        proj_fm(psq, zT, wq, qT, HTOK)
                        proj_fm(psq, zT, wk, kT, HTOK)
                    v = qp.tile([128, HALFW, C], BF16, tag="vt")

                    def v_proj(psv):
                        # emitted after the window-loop prologue: the 12us of
                        # PE work fills the softmax pipeline-fill bubble (v is
                        # first consumed by CD(0)'s AV matmuls)
                        for w in range(HALFW):
                            pv = psv.tile([128, 1024], F32, tag="po2")
                            for o, n in ((0, 512), (512, 128)):
                                for ci in range(CHUNKS):
                                    nc.tensor.matmul(
                                        pv[:, o:o + n],
                                        zT[:, ci, 128 * w:128 * (w + 1)],
                                        wv[:, ci, o:o + n],
                                        start=(ci == 0), stop=(ci == CHUNKS - 1))
                            evict(v[:, w, :], pv[:, 0:C], w=(2, 1, 1))
                    # rel-pos shear: s2byT[r, t, h, j] = q_{r,t}.rk[j-t+16]
                    s2byT = sp2.tile([HR, T, HEADS, 16], BF16, tag="s2byT")
                    with ExitStack() as hs:
                        psh = hs.enter_context(
                            tc.tile_pool(name="pshT", bufs=3, space="PSUM"))
                        for h in range(HEADS):
                            pSB = zp.tile([NREL, HTOK], BF16, tag="pSB")
                            for o in range(0, HTOK, 512):
                                n = min(512, HTOK - o)
                                pp = psh.tile([NREL, 512], F32, tag="pp")
                                nc.tensor.matmul(pp[:, 0:n], rkT[:],
                                                 qT[:, h, o:o + n],
                                                 start=True, stop=True)
                                evict(pSB[:, o:o + n], pp[:, 0:n], w=(1, 1, 1))
                            pSa = pSB[:, :]
                            sh = psh.tile([HR, T, 64], BF16, tag="sh")
                            for t in range(T):
                                src = bass.AP(
                                    tensor=pSB.tensor, offset=pSa.offset + t,
                                    ap=[list(pSa.ap[0]), [16, HR]])
                                nc.tensor.transpose(sh[:, t, 0:NREL], src,
                                                    identb[:NREL, :NREL])
                            # sheared copy: col j of (r,t) = sh[r, t, 16-t+j]
                            sha = sh[:, :, :]
                            s2a = s2byT[:, :, :, :]
                            src = bass.AP(
                                tensor=sh.tensor, offset=sha.offset + 16,
                                ap=[list(sha.ap[0]), [63, 16], [1, 16]])
                            dst = bass.AP(
                                tensor=s2byT.tensor,
                                offset=s2a.offset + 16 * h,
                                ap=[list(s2a.ap[0]), [HEADS * 16, 16], [1, 16]])
                            evict(dst, src, w=(1, 1, 1))
                        # bounce via DRAM: s2_dram[(72*half+r)*16+t, h, j]
                        s2flat = s2_dram[:]
                        d_dst = bass.AP(tensor=s2flat.tensor,
                                        offset=s2flat.offset + half * HR * 2048,
                                        ap=[[2048, HR], [1, 2048]])
                        s2a = s2byT[:, :, :, :]
                        d_src = bass.AP(tensor=s2byT.tensor, offset=s2a.offset,
                                        ap=[list(s2a.ap[0]), [1, 2048]])
                        nc.sync.dma_start(out=d_dst, in_=d_src)

                    # per-window attention, 3-deep pipeline:
                    # fa(w+2) scores; fb(w+1) softmax+AV; back(w) WO+resid
                    with ExitStack() as hs:
                        psA = hs.enter_context(
                            tc.tile_pool(name="psAT", bufs=2, space="PSUM"))
                        psB = hs.enter_context(
                            tc.tile_pool(name="psBT", bufs=2, space="PSUM"))
                        psC = hs.enter_context(
                            tc.tile_pool(name="psCT", bufs=1, space="PSUM"))

                        def t_fa(w):
                            wg = wlo + w
                            s2w = swp.tile([128, HEADS, 16], BF16, tag="s2w")
                            nc.sync.dma_start(
                                out=s2w[:], in_=s2_dram[128 * wg:128 * (wg + 1)])
                            # emask = mask * exp(scale*s2w), built on Act/Pool
                            # off the critical path
                            eb = swp.tile([128, HEADS, 16], BF16, tag="eb")
                            nc.scalar.activation(out=eb[:], in_=s2w[:],
                                                 func=AF.Exp, scale=SCALE)
                            em = swp.tile([128, HEADS, 128], BF16, tag="em")
                            eba = eb[:, :, :]
                            ebr = bass.AP(tensor=eb.tensor, offset=eba.offset,
                                          ap=[list(eba.ap[0]), [16, HEADS],
                                              [0, 8], [1, 16]])
                            maska = mask[:, :]
                            maskr = bass.AP(tensor=mask.tensor,
                                            offset=maska.offset,
                                            ap=[list(maska.ap[0]), [0, HEADS],
                                                [1, 128]])
                            nc.gpsimd.tensor_tensor(out=em[:], in0=maskr,
                                                    in1=ebr, op=ALU.mult)
                            pS = psA.tile([128, 1024], F32, tag="pS")
                            for h in range(HEADS):
                                nc.tensor.matmul(
                                    pS[:, 128 * h:128 * (h + 1)],
                                    qT[:, h, 128 * w:128 * (w + 1)],
                                    kT[:, h, 128 * w:128 * (w + 1)],
                                    start=True, stop=True)
                            return pS, em

        # stage AB: exp + mask + softmax stats + normalize (Act/DVE/Pool)
                        def t_AB(w, pS, em):
                            aG = swp.tile([128, HEADS, 128], BF16, tag="aG")
                            nc.scalar.activation(out=aG[:], in_=pS[:],
                                                 func=AF.Exp, scale=SCALE)
                            nc.vector.tensor_tensor(out=aG[:], in0=aG[:],
                                                    in1=em[:], op=ALU.mult)
                            aD = swp.tile([128, HEADS, 16], F32, tag="aD")
                            aGa = aG[:, :, :]
                            agv = bass.AP(tensor=aG.tensor, offset=aGa.offset,
                                          ap=[list(aGa.ap[0]), [128, HEADS],
                                              [1, 16], [16, 8]])
                            nc.vector.tensor_reduce(out=aD[:], in_=agv,
                                                    axis=AX.X, op=ALU.add)
                            zt = small.tile([128, HEADS], F32, tag="zt")
                            nc.vector.tensor_reduce(out=zt[:], in_=aD[:],
                                                    axis=AX.X, op=ALU.add)
                            nc.vector.reciprocal(out=zt[:], in_=zt[:])
                            zta = zt[:, :]
                            rzb = bass.AP(tensor=zt.tensor, offset=zta.offset,
                                          ap=[list(zta.ap[0]), [1, HEADS],
                                              [0, 128]])
                            nc.gpsimd.tensor_tensor(out=aG[:], in0=aG[:],
                                                    in1=rzb, op=ALU.mult)
                            if debug and prefix == "t1" and wlo + w == 0:
                                nc.sync.dma_start(out=dbg["aG"][:], in_=aG[:])
                                nc.sync.dma_start(out=dbg["v0"][:],
                                                  in_=v[:, 0, :])
                            rzb2 = bass.AP(tensor=zt.tensor, offset=zta.offset,
                                           ap=[list(zta.ap[0]), [1, HEADS],
                                               [0, 16]])
                            aDn = swp.tile([128, HEADS, 16], BF16, tag="aDn")
                            nc.gpsimd.tensor_tensor(out=aDn[:], in0=aD[:],
                                                    in1=rzb2, op=ALU.mult)
                            return aG, aDn

                        # stage CD: transposes + AV (+rel-V) + oT assembly
                        def t_CD(w, aG, aDn):
                            paT = psB.tile([128, 1024], BF16, tag="ptr")
                            for h in range(HEADS):
                                nc.tensor.transpose(
                                    paT[:, 128 * h:128 * (h + 1)], aG[:, h, :],
                                    identb[:])
                            aTs = swp.tile([128, HEADS, 128], BF16, tag="aTs")
                            nc.scalar.copy(out=aTs[:], in_=paT[:])
                            pdT = psB.tile([128, 1024], BF16, tag="ptr")
                            for h in range(HEADS):
                                nc.tensor.transpose(
                                    pdT[:16, 128 * h:128 * (h + 1)],
                                    aDn[:, h, :], identb[:])
                            aDT = swp.tile([16, HEADS, 128], BF16, tag="aDT")
                            nc.scalar.copy(out=aDT[:], in_=pdT[:16, :])
                            # o1 = v^T A (plain start/stop groups per slot)
                            pO = psA.tile([128, 1024], F32, tag="pS")
                            for h in range(HEADS):
                                nc.tensor.matmul(pO[:DH, 128 * h:128 * (h + 1)],
                                                 v[:, w, DH * h:DH * (h + 1)],
                                                 aTs[:, h, :],
                                                 start=True, stop=True)
                            # o2: disjoint strided cols, own psum, no accum;
                            # one 64-col matmul per frame t (all heads+rows)
                            pR = psC.tile([128, 1024], F32, tag="po2")
                            aDa = aDT[:, :, :]
                            pRa = pR[:, :]
                            # one 32-col matmul per (frame t, head-half hh):
                            # the 4-head span stays inside one psum bank
                            for t in range(T):
                                for hh in range(2):
                                    off = 512 * hh + t
                                    rhs = bass.AP(
                                        tensor=aDT.tensor,
                                        offset=aDa.offset + off,
                                        ap=[list(aDa.ap[0]), [128, 4],
                                            [16, 8]])
                                    ov = bass.AP(
                                        tensor=pR.tensor,
                                        offset=pRa.offset + off,
                                        ap=[[pRa.ap[0][0], DH], [128, 4],
                                            [16, 8]])
                                    nc.tensor.matmul(ov, rvs[:, t, :], rhs,
                                                     start=True, stop=True)
                            oT = swp.tile([DH, HEADS, 128], BF16, tag="oTt")
                            pOa = pO[:, :]
                            src0 = bass.AP(tensor=pO.tensor, offset=pOa.offset,
                                           ap=[[pOa.ap[0][0], DH], [128, HEADS],
                                               [1, 128]])
                            src1 = bass.AP(tensor=pR.tensor, offset=pRa.offset,
                                           ap=[[pRa.ap[0][0], DH], [128, HEADS],
                                               [1, 128]])
                            nc.scalar.copy(out=oT[:, :, :], in_=src0)
                            nc.vector.tensor_tensor(out=oT[:, :, :], in0=src1,
                                                    in1=oT[:, :, :], op=ALU.add)
                            if debug and prefix == "t1" and wlo + w == 0:
                                nc.sync.dma_start(out=dbg["oT0"][:], in_=oT[:])
                            return oT

                        def t_back(w, oT):
                            wg = wlo + w
                            wo_resid(psA, "pS", oT, 0, 128, wo, x_sb[:, wg, :])
                            if debug:
                                nc.sync.dma_start(
                                    out=dbg[dbg_key][:].rearrange(
                                        "r t c -> (r t) c")[128 * wg:128 * (wg + 1), :],
                                    in_=x_sb[:, wg, :])

                        # staged pipeline, emission order chosen so every
                        # engine queue is in ready order (in-order queues):
                        # AB(k+1); CD(k); fa(k+2); back(k)
                        fa_q, ab_q = {}, {}
                        fa_q[0] = t_fa(0)
                        fa_q[1] = t_fa(1)
                        ab_q[0] = t_AB(0, *fa_q.pop(0))
                        v_proj(psC)
                        for k in range(HALFW):
                            if k + 1 < HALFW:
                                ab_q[k + 1] = t_AB(k + 1, *fa_q.pop(k + 1))
                            oT = t_CD(k, *ab_q.pop(k))
                            if k + 2 < HALFW:
                                fa_q[k + 2] = t_fa(k + 2)
                            t_back(k, oT)

        _mark("phaseA")
        temporal("t1", "t1", filler=fill_half)
        _mark("t1")

        # =====================================================================
        # Cross-attention
        # =====================================================================
        with ExitStack() as ph:
            zp = ph.enter_context(tc.tile_pool(name="zpX", bufs=2))
            qp = ph.enter_context(tc.tile_pool(name="qpX", bufs=2))

            for half in range(2):
                wlo = half * HALFW
                zT = zp.tile([128, CHUNKS, HTOK], BF16, tag="zTx")
                qT = qp.tile([DH, HEADS, HTOK], BF16, tag="qx")
                with ExitStack() as hs:
                    psz = hs.enter_context(tc.tile_pool(name="pszX", bufs=2,
                                                        space="PSUM"))
                    ln_fm(psz, zp, lambda w: x_sb[:, wlo + w, :], zT, HALFW)
                    proj_fm(psz, zT, wqx, qT, HTOK)
                with ExitStack() as hs:
                    pss = hs.enter_context(tc.tile_pool(name="pssX", bufs=2,
                                                        space="PSUM"))
                    psB = hs.enter_context(tc.tile_pool(name="psBX", bufs=2,
                                                        space="PSUM"))
                    eS = qp.tile([77, HEADS, HTOK], BF16, tag="eSx")

                    def escores(o):
                        # eS chunk [o, o+n): only needed by windows >= o//128,
                        # so later chunks are emitted after the window loop
                        # starts — their Act-bound exps overlap window PE work
                        n = min(512, HTOK - o)
                        for h in range(HEADS):
                            ps = pss.tile([77, 512], F32, tag="psx")
                            nc.tensor.matmul(ps[:, 0:n], kctxT[:, h, :],
                                             qT[:, h, o:o + n],
                                             start=True, stop=True)
                            nc.scalar.activation(out=eS[:, h, o:o + n],
                                                 in_=ps[:, 0:n],
                                                 func=AF.Exp, scale=SCALE)

                    def x_fa(w):
                        oX = psB.tile([128, 1024], F32, tag="oX")
                        for h in range(HEADS):
                            nc.tensor.matmul(oX[:, 128 * h:128 * h + DH + 1],
                                             eS[:, h, 128 * w:128 * (w + 1)],
                                             v1x[:, h, :],
                                             start=True, stop=True)
                        return oX

                    def x_fb(w, oX):
                        rz = small.tile([128, HEADS], F32, tag="rzx")
                        oXa = oX[:, :]
                        zv = bass.AP(tensor=oX.tensor, offset=oXa.offset + DH,
                                     ap=[list(oXa.ap[0]), [128, HEADS]])
                        nc.vector.reciprocal(out=rz[:], in_=zv)
                        oN = zp.tile([128, HEADS, DH], BF16, tag="oNx")
                        src = bass.AP(tensor=oX.tensor, offset=oXa.offset,
                                      ap=[list(oXa.ap[0]), [128, HEADS],
                                          [1, DH]])
                        rza = rz[:, :]
                        rzb = bass.AP(tensor=rz.tensor, offset=rza.offset,
                                      ap=[list(rza.ap[0]), [1, HEADS], [0, DH]])
                        nc.vector.tensor_tensor(out=oN[:], in0=src, in1=rzb,
                                                op=ALU.mult)
                        pt = psB.tile([DH, HEADS, 128], BF16, tag="ptx")
                        for h in range(HEADS):
                            nc.tensor.transpose(pt[:, h, :], oN[:, h, :],
                                                identb[:])
                        oTx = zp.tile([DH, HEADS, 128], BF16, tag="oTx")
                        evict(oTx[:], pt[:], w=(2, 1, 0))
                        return oTx

                    def x_back(w, oTx):
                        wg = wlo + w
                        wo_resid(psB, "oX", oTx, 0, 128, wox, x_sb[:, wg, :])
                        if debug:
                            nc.sync.dma_start(
                                out=dbg["x2"][:].rearrange(
                                    "r t c -> (r t) c")[128 * wg:128 * (wg + 1), :],
                                in_=x_sb[:, wg, :])

                    # staged: fb(k+1) before fa(k+2) before back(k), so each
                    # engine queue stays in ready order; eS score chunks for
                    # later windows are emitted mid-loop
                    escores(0)
                    fa_q, fb_q = {}, {}
                    fa_q[0] = x_fa(0)
                    fa_q[1] = x_fa(1)
                    fb_q[0] = x_fb(0, fa_q.pop(0))
                    escores(512)
                    for k in range(HALFW):
                        if k + 1 < HALFW:
                            fb_q[k + 1] = x_fb(k + 1, fa_q.pop(k + 1))
                        if k + 2 < HALFW:
                            fa_q[k + 2] = x_fa(k + 2)
                        x_back(k, fb_q.pop(k))
                        if k == 2:
                            escores(1024)

        _mark("cross")
        temporal("t2", "t2")
        _mark("t2")

        # =====================================================================
        # GEGLU FFN per window. ff_w1 cols host-permuted into rounds of
        # (4 a-chunks, 4 gate-chunks); a-chunk order preserved for ff_w2.
        # =====================================================================
        with ExitStack() as ph:
            wp = ph.enter_context(tc.tile_pool(name="wpF", bufs=1))
            zp = ph.enter_context(tc.tile_pool(name="zpF", bufs=2))
            hp = ph.enter_context(tc.tile_pool(name="hpF", bufs=2))
            psp = ph.enter_context(tc.tile_pool(name="psF", bufs=2, space="PSUM"))
            psx = ph.enter_context(tc.tile_pool(name="psxF", bufs=1, space="PSUM"))
            psh = ph.enter_context(tc.tile_pool(name="pshF", bufs=2, space="PSUM"))

            w1 = wp.tile([128, CHUNKS, 2 * FFI], BF16, tag="w1")
            nc.sync.dma_start(out=w1[:],
                              in_=wts["ff_w1"][:].rearrange("(a p) n -> p a n",
                                                            p=128))
            w2 = wp.tile([128, NG2, C], BF16, tag="w2")
            nc.sync.dma_start(out=w2[:],
                              in_=wts["ff_w2"][:].rearrange("(a p) n -> p a n",
                                                            p=128))

            stats = ln_stats(wp, lambda w: x_sb[:, w, :], NWIN, "lnF")
            for sw in range(5):
                w0 = 4 * sw
                nw = 4 if sw < 4 else 2
                ntok = 128 * nw
                zT = zp.tile([128, CHUNKS, 512], BF16, tag="zTf")
                ln_fm(psp, zp, lambda i: x_sb[:, w0 + i, :], zT, nw,
                      stats=stats, w0=w0)
                uT = hp.tile([128, NG2, 512], BF16, tag="uT")
                for r in range(5):
                    for p in range(4):
                        ph_ = psh.tile([128, 2, 512], F32, tag="ph")
                        for j, co in ((0, p), (1, 4 + p)):
                            gcol = 1024 * r + 128 * co
                            for ci in range(CHUNKS):
                                nc.tensor.matmul(ph_[:, j, 0:ntok],
                                                 w1[:, ci, gcol:gcol + 128],
                                                 zT[:, ci, 0:ntok],
                                                 start=(ci == 0),
                                                 stop=(ci == CHUNKS - 1))
                        gl = hp.tile([128, 512], BF16, tag="gelu")
                        nc.scalar.activation(out=gl[:, 0:ntok],
                                             in_=ph_[:, 1, 0:ntok],
                                             func=AF.Gelu)
                        nc.vector.tensor_tensor(out=uT[:, 4 * r + p, 0:ntok],
                                                in0=ph_[:, 0, 0:ntok],
                                                in1=gl[:, 0:ntok],
                                                op=ALU.mult)
                for i in range(nw):
                    w = w0 + i
                    px = psx.tile([128, 1024], F32, tag="px")
                    for o, n in ((0, 512), (512, 128)):
                        for ci in range(NG2):
                            nc.tensor.matmul(px[:, o:o + n],
                                             uT[:, ci, 128 * i:128 * (i + 1)],
                                             w2[:, ci, o:o + n],
                                             start=(ci == 0), stop=(ci == NG2 - 1))
                    xout = zp.tile([128, C], F32, tag="xout")
                    nc.vector.scalar_tensor_tensor(
                        out=xout[:], in0=px[:, 0:C], scalar=1.0,
                        in1=x_sb[:, w, :], op0=ALU.mult, op1=ALU.add)
                    nc.sync.dma_start(
                        out=out_final[:].rearrange(
                            "r t c -> (r t) c")[128 * w:128 * (w + 1), :],
                        in_=xout[:])

    _mark("ffn")
    bass.BassTensorEngine.matmul = _omm
    if not nc.is_finalized():
        nc.finalize()
    return nc


# ----------------------------------------------------------------------------
# host side
# ----------------------------------------------------------------------------

def _bf(a):
    return np.asarray(a, dtype=ml_dtypes.bfloat16)


def prepare_inputs(inputs):
    f = {k: np.asarray(v, dtype=np.float32) for k, v in inputs.items()}
    shared = {}

    def fold(g, b, wname):
        wf = f[wname]
        bias = f[b] @ wf
        assert np.abs(bias).max() < 1e-6, f"nonzero folded bias for {wname}"
        return f[g][:, None] * wf

    for k in ("a1_bo", "a2_bo", "t1_bo", "t2_bo", "ff_b1", "ff_b2"):
        assert np.abs(f[k]).max() < 1e-6, f"nonzero bias {k} unsupported"

    for p, gk, bk_ in (("a1", "g1", "b1"), ("t1", "g4", "b4"),
                       ("t2", "g5", "b5")):
        for kind in ("wq", "wk", "wv"):
            shared[f"{p}_{kind}"] = _bf(fold(gk, bk_, f"{p}_{kind}"))
    shared["a2_wq"] = _bf(fold("g2", "b2", "a2_wq"))
    shared["a2_wk"] = _bf(f["a2_wk"])
    shared["a2_wv"] = _bf(f["a2_wv"])
    for p in ("a1", "a2", "t1", "t2"):
        shared[f"{p}_wo"] = _bf(
            f[f"{p}_wo"].reshape(HEADS, DH, C).transpose(1, 0, 2))
    for p in ("t1", "t2"):
        shared[f"{p}_rkT"] = _bf(f[f"{p}_rk"].T)
        rv = f[f"{p}_rv"]
        rvs = np.zeros((16, T, DH), np.float32)
        for t in range(T):
            for j in range(16):
                rvs[j, t] = rv[j - t + MAXREL]
        shared[f"{p}_rvs"] = _bf(rvs)
    w1f = fold("g3", "b3", "ff_w1")
    a_, g_ = w1f[:, :FFI], w1f[:, FFI:]
    cols = []
    for r in range(5):
        cols.append(a_[:, 512 * r:512 * (r + 1)])
        cols.append(g_[:, 512 * r:512 * (r + 1)])
    shared["ff_w1"] = _bf(np.concatenate(cols, axis=1))
    shared["ff_w2"] = _bf(f["ff_w2"])
    m = np.zeros((128, 128), np.float32)
    for g in range(8):
        m[16 * g:16 * (g + 1), 16 * g:16 * (g + 1)] = 1.0
    shared["bd_mask"] = _bf(m)

    x = f["x"]
    ctx = f["context"]
    in_maps = []
    for core in range(NCORES):
        im = dict(shared)
        xs = np.empty((NG, SEQ, C), np.float32)
        for g in range(NG):
            bt = core + 8 * g
            b, t = bt // T, bt % T
            xs[g] = x[b, :, t].reshape(C, SEQ).T
        im["xs_in"] = _bf(xs)
        im["ctxT"] = _bf(ctx[core // 4].T.copy())
        in_maps.append(im)
    return in_maps


_PROGRAM_CACHE = {}


def run(inputs, debug=False, trace=False):
    key = "dbg" if debug else "plain"
    if key not in _PROGRAM_CACHE:
        _PROGRAM_CACHE[key] = build_program(debug=debug)
    nc = _PROGRAM_CACHE[key]
    in_maps = prepare_inputs(inputs)
    from concourse.bass_utils import run_bass_kernel_spmd
    res = run_bass_kernel_spmd(nc, in_maps, list(range(NCORES)), trace=trace)
    outs = res.results
    full = np.empty((B * H * W, T, C), np.float32)
    for core in range(NCORES):
        full[NR * core:NR * (core + 1)] = outs[core]["out"]
    y = full.reshape(B, H, W, T, C).transpose(0, 4, 3, 1, 2)
    return y, res, outs


def kernel(**inputs):
    y, _, _ = run(inputs)
    return y.astype(np.float32)



# revision 114
# speedup vs baseline: 1.0010x; 1.0010x over previous
"""Trainium2 Bass kernel for BasicTransformerBlockST (spatial/temporal block).

Sharding over 8 NeuronCores (same as baseline):
  Phase A (spatial self-attn): data-parallel over (b,t): core i owns the 4
  groups bt = i + 8g. An 8-way AllToAll (split in two, overlapped with phase
  A compute) reshards to (b,h,w)-parallel: core j owns rows
  (b=j//4, hw in [144*(j%4), 144*(j%4+1))), tokens r-major (token = r*16+t).
  t1 / cross-attn / t2 / FFN run on that shard with the residual stream
  resident in SBUF (no DRAM bounces).

Optimized for the TimelineSim cost model: batched big instructions, S^T
softmax formulation (no attention-matrix transposes or renormalize in phase
A / cross), z via ones-column fused into AV, evictions spread across
DVE/Act/Pool, PSUM tag sharing for double buffering.

Scheduling notes (engines execute their queues strictly in order, so
emission order is the schedule):
 - residual crosses the AllToAll in bf16; each slot is split into row-half
   collectives in separate DRAM tensors so t1 half-0 starts during the
   second collective
 - temporal/cross window loops are staged pipelines emitted in per-engine
   ready order (AB(k+1); CD(k); fa(k+2); back(k))
 - V projection is emitted after the window-loop prologue so its PE work
   fills the softmax pipeline-fill bubble
 - LN stats are batched per phase (one Sqrt act-table episode each)
 - a single matmul's PSUM output span must stay inside one 2KB bank
   (the o2 rel-V matmuls are split per head-half for this); accumulating
   with start=False onto a region written by a different matmul shape
   produces wrong results on this stack - keep o1/o2 in separate psum
"""

import sys

sys.path.insert(0, "/opt/trn_rl_repo")

import numpy as np
import ml_dtypes

import concourse.bass as bass
import concourse.bacc as bacc
import concourse.mybir as mybir
import concourse.tile as tile
from concourse.masks import make_identity

F32 = mybir.dt.float32
BF16 = mybir.dt.bfloat16
AF = mybir.ActivationFunctionType
ALU = mybir.AluOpType
AX = mybir.AxisListType

B, C, T, H, W = 2, 640, 16, 24, 24
HEADS, DH = 8, 80
CTXD = 1024
MAXREL = 16
NREL = 2 * MAXREL + 1          # 33
FFI = 4 * C                    # 2560
INNER = HEADS * DH             # 640
SCALE = DH ** -0.5
EPS = 1e-5

NCORES = 8
NG = 4                         # spatial groups per core
SEQ = H * W                    # 576
NR = (B * H * W) // NCORES     # 144 rows per core
TOK = NR * T                   # 2304 tokens per core
NWIN = TOK // 128              # 18
CHUNKS = C // 128              # 5
CTXCH = CTXD // 128            # 8
HALFW = NWIN // 2              # 9 windows per temporal half
HR = NR // 2                   # 72 rows per half
HTOK = 128 * HALFW             # 1152 tokens per half
NG2 = FFI // 128               # 20 ffn chunks

# token chunks of a 576-token spatial group
QSP = [(0, 128), (128, 128), (256, 128), (384, 128), (512, 64)]


PHASE_MARKS = []


def build_program(debug=False):
    nc = bacc.Bacc(None, target_bir_lowering=False)

    # instrument PE-instruction counts at phase boundaries (analysis only)
    PHASE_MARKS.clear()
    _mmcnt = [0]
    _omm = bass.BassTensorEngine.matmul

    def _cmm(self, *a, **k):
        _mmcnt[0] += 1
        return _omm(self, *a, **k)

    bass.BassTensorEngine.matmul = _cmm

    def _mark(name):
        PHASE_MARKS.append((name, _mmcnt[0]))

    xs_in = nc.dram_tensor("xs_in", [NG, SEQ, C], BF16, kind="ExternalInput")
    ctxT_in = nc.dram_tensor("ctxT", [CTXD, 77], BF16, kind="ExternalInput")

    def win(name, shape, dt=BF16):
        return nc.dram_tensor(name, shape, dt, kind="ExternalInput")

    wts = {}
    for p in ("a1", "a2", "t1", "t2"):
        cin = CTXD if p == "a2" else C
        wts[f"{p}_wq"] = win(f"{p}_wq", [C, INNER])
        wts[f"{p}_wk"] = win(f"{p}_wk", [cin, INNER])
        wts[f"{p}_wv"] = win(f"{p}_wv", [cin, INNER])
        wts[f"{p}_wo"] = win(f"{p}_wo", [DH, HEADS, C])
    for p in ("t1", "t2"):
        wts[f"{p}_rkT"] = win(f"{p}_rkT", [DH, NREL])
        wts[f"{p}_rvs"] = win(f"{p}_rvs", [16, T, DH])  # rvs[j,t,d]=rv[j-t+16,d]
    wts["ff_w1"] = win("ff_w1", [C, 2 * FFI])  # host-permuted cols (4a,4g)
    wts["ff_w2"] = win("ff_w2", [FFI, C])
    bd_mask = win("bd_mask", [128, 128], BF16)

    out_final = nc.dram_tensor("out", [NR, T, C], F32, kind="ExternalOutput")
    dbg = {}
    if debug:
        dbg["a"] = nc.dram_tensor("dbg_a", [NG, SEQ, C], BF16,
                                  kind="ExternalOutput")
        for nm in ("t1", "x2", "t2"):
            dbg[nm] = nc.dram_tensor(f"dbg_{nm}", [NR, T, C], F32,
                                     kind="ExternalOutput")
        dbg["aG"] = nc.dram_tensor("dbg_aG", [128, HEADS, 128], BF16,
                                   kind="ExternalOutput")
        dbg["v0"] = nc.dram_tensor("dbg_v0", [128, C], BF16,
                                   kind="ExternalOutput")
        dbg["q0"] = nc.dram_tensor("dbg_q0", [DH, HEADS, 128], BF16,
                                   kind="ExternalOutput")
        dbg["oT0"] = nc.dram_tensor("dbg_oT0", [DH, HEADS, 128], BF16,
                                    kind="ExternalOutput")

    # slot-major a2a: slot s holds frames t = i + 8*s from src core i.
    # One tensor per row half (rh) so the temporal phase's half-0 fill
    # only depends on the rh=0 collectives; slot is the leading dim.
    a2a_in = [nc.dram_tensor(f"a2a_in{r}", [2, NCORES, HR, C], BF16)
              for r in range(2)]
    a2a_out = [nc.dram_tensor(f"a2a_out{r}", [2, NCORES, HR, C], BF16)
               for r in range(2)]
    s2_dram = nc.dram_tensor("s2_dram", [TOK, HEADS, 16], BF16)
    groups = [[0, 1, 2, 3, 4, 5, 6, 7]]

    from contextlib import ExitStack

    with tile.TileContext(nc) as tc, ExitStack() as top:
        const = top.enter_context(tc.tile_pool(name="const", bufs=1))
        identb = const.tile([128, 128], BF16)
        make_identity(nc, identb)
        eps_t = const.tile([128, 1], F32)
        nc.vector.memset(eps_t[:], EPS)
        mask = const.tile([128, 128], BF16)
        nc.sync.dma_start(out=mask[:], in_=bd_mask[:, :])
        small = top.enter_context(tc.tile_pool(name="small", bufs=6))
        resp = top.enter_context(tc.tile_pool(name="resp", bufs=1))
        x_sb = resp.tile([128, NWIN, C], F32, tag="x_sb")

        ev_state = [0]

        def evict(out, in_, w=(1, 1, 1)):
            """psum->sbuf copy via rotating engines; w=(dve, act, act2).
            GPSIMD cannot access PSUM, so only DVE/Act here."""
            seq = [nc.vector] * w[0] + [nc.scalar] * (w[1] + w[2])
            eng = seq[ev_state[0] % len(seq)]
            ev_state[0] += 1
            if eng is nc.scalar:
                eng.copy(out=out, in_=in_)
            else:
                eng.tensor_copy(out=out, in_=in_)

        def ln_stats(sp, xfn, nw, tag):
            """Batched LN stats: one Sqrt activation for all nw windows so
            the Act engine swaps function tables once per phase, not per
            window. Returns (mv [128,nw,2], rstd [128,nw])."""
            mv = sp.tile([128, nw, 2], F32, tag=tag + "_mv")
            for w in range(nw):
                x = xfn(w)
                st = small.tile([128, 2, 6], F32, tag="bnst")
                nc.vector.bn_stats(out=st[:, 0, :], in_=x[:, 0:512])
                nc.vector.bn_stats(out=st[:, 1, :], in_=x[:, 512:640])
                nc.vector.bn_aggr(out=mv[:, w, :], in_=st[:])
            rstd = sp.tile([128, nw], F32, tag=tag + "_rs")
            mva = mv[:, :, :]
            var = bass.AP(tensor=mv.tensor, offset=mva.offset + 1,
                          ap=[list(mva.ap[0]), [2, nw]])
            nc.scalar.activation(out=rstd[:], in_=var, func=AF.Sqrt,
                                 bias=eps_t[:], scale=1.0)
            nc.vector.reciprocal(out=rstd[:], in_=rstd[:])
            return mv, rstd

        def ln_fm(psp, zp, xfn, zT, nw, stats=None, w0=0):
            """LayerNorm (g/b folded into weights) + transpose into
            feature-major zT[:, ci, 128*w : 128*w+128] bf16."""
            zT_a = zT[:, :, :]
            ntok = zT_a.ap[1][0]
            if stats is None:
                stats = ln_stats(zp, xfn, nw, "lnf")
            mv, rstd = stats
            for w in range(nw):
                x = xfn(w)
                zs = zp.tile([128, C], BF16, tag="zs")
                nc.vector.tensor_scalar(
                    out=zs[:], in0=x, scalar1=mv[:, w0 + w, 0:1],
                    scalar2=rstd[:, w0 + w:w0 + w + 1],
                    op0=ALU.subtract, op1=ALU.mult)
                pz = psp.tile([128, CHUNKS, 128], BF16, tag="pz")
                for c in range(CHUNKS):
                    nc.tensor.transpose(pz[:, c, :], zs[:, 128 * c:128 * (c + 1)],
                                        identb[:])
                dst = bass.AP(tensor=zT.tensor,
                              offset=zT_a.offset + 128 * w,
                              ap=[list(zT_a.ap[0]), [ntok, CHUNKS], [1, 128]])
                evict(dst, pz[:, :, :], w=(2, 1, 1))

        def load_w_cin(wp, name, cin):
            t = wp.tile([128, cin // 128, wts[name].shape[-1]], BF16,
                        tag="w_" + name)
            nc.sync.dma_start(out=t[:],
                              in_=wts[name][:].rearrange("(a p) n -> p a n", p=128))
            return t

        def load_wo(wp, name):
            t = wp.tile([DH, HEADS, C], BF16, tag="w_" + name)
            nc.sync.dma_start(out=t[:], in_=wts[name][:])
            return t

        def proj_fm(psp, zT, w_sb, qT, ntok):
            """feature-major projection qT[80, h, ntok] (bf16).
            PSUM allocations cap at 4KB, so one 1-bank tile per 512-split."""
            for h in range(HEADS):
                for o in range(0, ntok, 512):
                    n = min(512, ntok - o)
                    pq = psp.tile([128, 512], F32, tag="pA")
                    for ci in range(CHUNKS):
                        nc.tensor.matmul(pq[:DH, 0:n],
                                         w_sb[:, ci, DH * h:DH * (h + 1)],
                                         zT[:, ci, o:o + n],
                                         start=(ci == 0), stop=(ci == CHUNKS - 1))
                    evict(qT[:, h, o:o + n], pq[:DH, 0:n], w=(2, 2, 1))

        def wo_resid(psp, tag, oT, qoff, ntok, wo_sb, resid_ap):
            """WO projection (by-head lhsT oT[:, h, qoff:qoff+ntok]) +
            residual add into resid_ap [ntok, C]."""
            pw = psp.tile([128, 1024], F32, tag=tag)
            for o, n in ((0, 512), (512, 128)):
                for h in range(HEADS):
                    nc.tensor.matmul(pw[:ntok, o:o + n],
                                     oT[:, h, qoff:qoff + ntok],
                                     wo_sb[:, h, o:o + n],
                                     start=(h == 0), stop=(h == HEADS - 1))
            nc.vector.scalar_tensor_tensor(
                out=resid_ap, in0=pw[:ntok, 0:C], scalar=1.0, in1=resid_ap,
                op0=ALU.mult, op1=ALU.add)

        # =====================================================================
        # PHASE A: spatial self-attention per (b,t) group; order 0,2,1,3 so
        # each a2a slot's collective fires after two groups.
        # =====================================================================
        with ExitStack() as ph:
            xap = ph.enter_context(tc.tile_pool(name="xapA", bufs=1))
            wp = ph.enter_context(tc.tile_pool(name="wpA", bufs=1))
            zp = ph.enter_context(tc.tile_pool(name="zpA", bufs=2))
            qp = ph.enter_context(tc.tile_pool(name="qpA", bufs=2))
            ap_ = ph.enter_context(tc.tile_pool(name="apA", bufs=2))
            psp = ph.enter_context(tc.tile_pool(name="psA", bufs=3, space="PSUM"))
            pso = ph.enter_context(tc.tile_pool(name="psoA", bufs=2, space="PSUM"))

            # all 4 groups' inputs DMA'd up front (before the weight loads in
            # the DMA queue) + one batched LN-stats pass: a single Sqrt table
            # episode for the whole phase instead of one per group
            xall = xap.tile([128, NG, CHUNKS, C], BF16, tag="xall")
            for g in (0, 2, 1, 3):
                nc.sync.dma_start(out=xall[:, g, 0:4, :],
                                  in_=xs_in[g, 0:512, :].rearrange(
                                      "(a p) c -> p a c", p=128))
                nc.sync.dma_start(out=xall[:64, g, 4, :],
                                  in_=xs_in[g, 512:576, :])
            statsA = ln_stats(wp, lambda k: xall[:, k // 5, k % 5, :],
                              NG * CHUNKS, "lnA")

            wq = load_w_cin(wp, "a1_wq", C)
            wk = load_w_cin(wp, "a1_wk", C)
            wv = load_w_cin(wp, "a1_wv", C)
            wo = load_wo(wp, "a1_wo")

            for g in (0, 2, 1, 3):
                zT = zp.tile([128, CHUNKS, 640], BF16, tag="zTa")
                ln_fm(pso, zp, lambda w, g=g: xall[:, g, w, :], zT, 5,
                      stats=statsA, w0=5 * g)

                qT = qp.tile([DH, HEADS, SEQ], BF16, tag="qa")
                kT = qp.tile([DH, HEADS, SEQ], BF16, tag="ka")
                proj_fm(psp, zT, wq, qT, SEQ)
                proj_fm(psp, zT, wk, kT, SEQ)

                # v token-major with ones column per head (memset 1.0 first;
                # the projection evictions overwrite all but the ones column)
                v1 = qp.tile([128, CHUNKS, HEADS, DH + 1], BF16, tag="va")
                nc.gpsimd.memset(v1[:], 1.0)
                for (w, (o_, np_)) in enumerate(QSP):
                    pv = psp.tile([128, 1024], F32, tag="pA")
                    for o, n in ((0, 512), (512, 128)):
                        for ci in range(CHUNKS):
                            nc.tensor.matmul(pv[:np_, o:o + n],
                                             zT[:, ci, o_:o_ + np_],
                                             wv[:, ci, o:o + n],
                                             start=(ci == 0), stop=(ci == CHUNKS - 1))
                    v1a = v1[:, :, :, :]
                    dst = bass.AP(tensor=v1.tensor,
                                  offset=v1a.offset + w * HEADS * (DH + 1),
                                  ap=[[v1a.ap[0][0], np_], [DH + 1, HEADS],
                                      [1, DH]])
                    evict(dst, pv[:np_, 0:C], w=(2, 1, 1))

                oT = ap_.tile([DH, HEADS, SEQ], BF16, tag="oa")

                def a_front(h):
                    """scores exp(S^T) for head h"""
                    eS = ap_.tile([128, CHUNKS, SEQ], BF16, tag="eS")
                    for (kc, (ko, kp)) in enumerate(QSP):
                        ps = psp.tile([128, 1024], F32, tag="pA")
                        for o, n in ((0, 512), (512, 64)):
                            nc.tensor.matmul(ps[:kp, o:o + n],
                                             kT[:, h, ko:ko + kp],
                                             qT[:, h, o:o + n],
                                             start=True, stop=True)
                        nc.scalar.activation(out=eS[:kp, kc, 0:SEQ],
                                             in_=ps[:kp, 0:SEQ],
                                             func=AF.Exp, scale=SCALE)
                    return eS

                def a_back(h, eS):
                    # AV + z via ones column: oA[q, 80] = z
                    oA = pso.tile([128, CHUNKS, 96], F32, tag="pz")
                    for (qc, (qo, qp_)) in enumerate(QSP):
                        for (kc, (ko, kp)) in enumerate(QSP):
                            nc.tensor.matmul(oA[:qp_, qc, 0:DH + 1],
                                             eS[:kp, kc, qo:qo + qp_],
                                             v1[:kp, kc, h, :],
                                             start=(kc == 0), stop=(kc == 4))
                    rz = small.tile([128, CHUNKS], F32, tag="rz")
                    oAa = oA[:, :, :]
                    zv = bass.AP(tensor=oA.tensor, offset=oAa.offset + DH,
                                 ap=[list(oAa.ap[0]), [96, CHUNKS]])
                    nc.vector.reciprocal(out=rz[:], in_=zv)
                    oN = ap_.tile([128, CHUNKS, DH], BF16, tag="oN")
                    src = bass.AP(tensor=oA.tensor, offset=oAa.offset,
                                  ap=[list(oAa.ap[0]), [96, CHUNKS], [1, DH]])
                    rza = rz[:, :]
                    rzb = bass.AP(tensor=rz.tensor, offset=rza.offset,
                                  ap=[list(rza.ap[0]), [1, CHUNKS], [0, DH]])
                    nc.vector.tensor_tensor(out=oN[:], in0=src, in1=rzb,
                                            op=ALU.mult)
                    pt = pso.tile([DH, CHUNKS, 128], BF16, tag="pz")
                    for (qc, (qo, qp_)) in enumerate(QSP):
                        nc.tensor.transpose(pt[:, qc, 0:qp_], oN[:qp_, qc, :],
                                            identb[:qp_, :qp_])
                    pta = pt[:, :, :]
                    src = bass.AP(tensor=pt.tensor, offset=pta.offset,
                                  ap=[list(pta.ap[0]), [128, 4], [1, 128]])
                    evict(oT[:, h, 0:512], src, w=(2, 1, 1))
                    evict(oT[:, h, 512:576], pt[:, 4, 0:64], w=(2, 1, 1))

                # software-pipeline heads: scores(h+1) before AV/norm(h) so
                # the PE never waits on head h's exp chain
                prev_eS = None
                for h in range(HEADS):
                    eS = a_front(h)
                    if prev_eS is not None:
                        a_back(h - 1, prev_eS)
                    prev_eS = eS
                a_back(HEADS - 1, prev_eS)

                # WO + residual (in place on xg), cast to bf16 for the
                # collective, scatter to a2a_in
                b_, tslot = g // 2, g % 2
                for (qc, (qo, qp_)) in enumerate(QSP):
                    xq = xall[:qp_, g, qc, :]
                    wo_resid(psp, "pA", oT, qo, qp_, wo, xq)
                    q0, q1 = qo // HR, (qo + qp_ - 1) // HR
                    for q in range(q0, q1 + 1):
                        lo, hi = max(qo, HR * q), min(qo + qp_, HR * (q + 1))
                        nc.sync.dma_start(
                            out=a2a_in[q % 2][tslot, 4 * b_ + q // 2,
                                              lo - HR * q:hi - HR * q, :],
                            in_=xall[lo - qo:hi - qo, g, qc, :])
                    if debug:
                        nc.sync.dma_start(out=dbg["a"][g, qo:qo + qp_, :],
                                          in_=xall[:qp_, g, qc, :])
                if g == 2:
                    for r in range(2):
                        nc.gpsimd.collective_compute(
                            "AllToAll", ALU.bypass, replica_groups=groups,
                            ins=[a2a_in[r][0]], outs=[a2a_out[r][0]])
            for r in range(2):
                nc.gpsimd.collective_compute(
                    "AllToAll", ALU.bypass, replica_groups=groups,
                    ins=[a2a_in[r][1]], outs=[a2a_out[r][1]])

        # cross-attention KV setup hoisted here: it has no dependency on
        # the AllToAll, so PE/DMA work lands inside the collective gap
        xkv = top.enter_context(tc.tile_pool(name="xkv", bufs=1))
        wqx = load_w_cin(xkv, "a2_wq", C)
        wox = load_wo(xkv, "a2_wo")
        with ExitStack() as hs:
            kvp = hs.enter_context(tc.tile_pool(name="kvpX", bufs=1))
            psk = hs.enter_context(tc.tile_pool(name="pskX", bufs=2,
                                                space="PSUM"))
            wkc = load_w_cin(kvp, "a2_wk", CTXD)
            wvc = load_w_cin(kvp, "a2_wv", CTXD)
            ctx_sb = kvp.tile([128, CTXCH, 77], BF16, tag="ctx")
            nc.sync.dma_start(out=ctx_sb[:],
                              in_=ctxT_in[:].rearrange("(a p) m -> p a m",
                                                       p=128))
            kctxT = xkv.tile([DH, HEADS, 77], BF16, tag="kctx")
            pk = psk.tile([DH, HEADS, 128], F32, tag="pk")
            for h in range(HEADS):
                for ci in range(CTXCH):
                    nc.tensor.matmul(pk[:, h, 0:77],
                                     wkc[:, ci, DH * h:DH * (h + 1)],
                                     ctx_sb[:, ci, :],
                                     start=(ci == 0), stop=(ci == CTXCH - 1))
            pka = pk[:, :, :]
            src = bass.AP(tensor=pk.tensor, offset=pka.offset,
                          ap=[list(pka.ap[0]), [128, HEADS], [1, 77]])
            evict(kctxT[:, :, :], src, w=(1, 1, 1))
            v1x = xkv.tile([77, HEADS, DH + 1], BF16, tag="vctx")
            nc.gpsimd.memset(v1x[:], 1.0)
            pv = psk.tile([77, 1024], F32, tag="pvx")
            for o, n in ((0, 512), (512, 128)):
                for ci in range(CTXCH):
                    nc.tensor.matmul(pv[:, o:o + n], ctx_sb[:, ci, :],
                                     wvc[:, ci, o:o + n],
                                     start=(ci == 0), stop=(ci == CTXCH - 1))
            v1a = v1x[:, :, :]
            dst = bass.AP(tensor=v1x.tensor, offset=v1a.offset,
                          ap=[list(v1a.ap[0]), [DH + 1, HEADS], [1, DH]])
            evict(dst, pv[:, 0:C], w=(1, 1, 1))

        # fill x_sb windows from a2a_out: partition p=16r'+t, t=i+8s;
        # bf16 staging + per-window upcast back to the f32 residual.
        # Done per temporal half (scoped pool) so half-0's pipeline never
        # waits on the half-1 collective.
        def fill_half(half, pool):
            xst = pool.tile([128, HALFW, C], BF16, tag="xst")
            base = a2a_out[half][:]
            for i in range(HALFW):
                src = bass.AP(tensor=base.tensor,
                              offset=base.offset + 8 * i * C,
                              ap=[[C, 8], [NCORES * HR * C, 2],
                                  [HR * C, 8], [1, C]])
                nc.sync.dma_start(out=xst[:, i, :], in_=src)
                w = half * HALFW + i
                if i % 3 == 0:
                    nc.vector.tensor_copy(out=x_sb[:, w, :],
                                          in_=xst[:, i, :])
                elif i % 3 == 1:
                    nc.scalar.copy(out=x_sb[:, w, :], in_=xst[:, i, :])
                else:
                    nc.gpsimd.tensor_copy(out=x_sb[:, w, :],
                                          in_=xst[:, i, :])

        # =====================================================================
        # Temporal attention (t1 / t2), per half
        # =====================================================================
        def temporal(prefix, dbg_key, filler=None):
            with ExitStack() as ph:
                wp = ph.enter_context(tc.tile_pool(name="wpT", bufs=1))
                zp = ph.enter_context(tc.tile_pool(name="zpT", bufs=2))
                ztp = ph.enter_context(tc.tile_pool(name="ztpT", bufs=1))
                qp = ph.enter_context(tc.tile_pool(name="qpT", bufs=1))
                swp = ph.enter_context(tc.tile_pool(name="swpT", bufs=2))
                sp2 = ph.enter_context(tc.tile_pool(name="sp2T", bufs=2))

                wq = load_w_cin(wp, f"{prefix}_wq", C)
                wk = load_w_cin(wp, f"{prefix}_wk", C)
                wv = load_w_cin(wp, f"{prefix}_wv", C)
                wo = load_wo(wp, f"{prefix}_wo")
                rkT = wp.tile([DH, NREL], BF16, tag="rkT")
                nc.sync.dma_start(out=rkT[:], in_=wts[f"{prefix}_rkT"][:])
                rvs = wp.tile([16, T, DH], BF16, tag="rvs")
                nc.sync.dma_start(out=rvs[:], in_=wts[f"{prefix}_rvs"][:])

                for half in range(2):
                    if filler is not None:
                        filler(half, qp)
                    wlo = half * HALFW
                    zT = ztp.tile([128, CHUNKS, HTOK], BF16, tag="zTt")
                    with ExitStack() as hs:
                        psz = hs.enter_context(
                            tc.tile_pool(name="pszT", bufs=4, space="PSUM"))
                        ln_fm(psz, zp, lambda w: x_sb[:, wlo + w, :], zT, HALFW)
                    qT = qp.tile([DH, HEADS, HTOK], BF16, tag="qt")
                    kT = qp.tile([DH, HEADS, HTOK], BF16, tag="kt")
                    with ExitStack() as hs:
                        psq = hs.enter_context(
                            tc.tile_pool(name="psqT", bufs=6, space="PSUM"))
                        proj_fm(psq, zT, wq, qT, HTOK)
                        proj_fm(psq, zT, wk, kT, HTOK)
                    v = qp.tile([128, HALFW, C], BF16, tag="vt")

                    def v_proj(psv):
                        # emitted after the window-loop prologue: the 12us of
                        # PE work fills the softmax pipeline-fill bubble (v is
                        # first consumed by CD(0)'s AV matmuls)
                        for w in range(HALFW):
                            pv = psv.tile([128, 1024], F32, tag="po2")
                            for o, n in ((0, 512), (512, 128)):
                                for ci in range(CHUNKS):
                                    nc.tensor.matmul(
                                        pv[:, o:o + n],
                                        zT[:, ci, 128 * w:128 * (w + 1)],
                                        wv[:, ci, o:o + n],
                                        start=(ci == 0), stop=(ci == CHUNKS - 1))
                            evict(v[:, w, :], pv[:, 0:C], w=(2, 1, 1))
                    # rel-pos shear: s2byT[r, t, h, j] = q_{r,t}.rk[j-t+16]
                    s2byT = sp2.tile([HR, T, HEADS, 16], BF16, tag="s2byT")
                    with ExitStack() as hs:
                        psh = hs.enter_context(
                            tc.tile_pool(name="pshT", bufs=3, space="PSUM"))
                        for h in range(HEADS):
                            pSB = zp.tile([NREL, HTOK], BF16, tag="pSB")
                            for o in range(0, HTOK, 512):
                                n = min(512, HTOK - o)
                                pp = psh.tile([NREL, 512], F32, tag="pp")
                                nc.tensor.matmul(pp[:, 0:n], rkT[:],
                                                 qT[:, h, o:o + n],
                                                 start=True, stop=True)
                                evict(pSB[:, o:o + n], pp[:, 0:n], w=(1, 1, 1))
                            pSa = pSB[:, :]
                            sh = psh.tile([HR, T, 64], BF16, tag="sh")
                            for t in range(T):
                                src = bass.AP(
                                    tensor=pSB.tensor, offset=pSa.offset + t,
                                    ap=[list(pSa.ap[0]), [16, HR]])
                                nc.tensor.transpose(sh[:, t, 0:NREL], src,
                                                    identb[:NREL, :NREL])
                            # sheared copy: col j of (r,t) = sh[r, t, 16-t+j]
                            sha = sh[:, :, :]
                            s2a = s2byT[:, :, :, :]
                            src = bass.AP(
                                tensor=sh.tensor, offset=sha.offset + 16,
                                ap=[list(sha.ap[0]), [63, 16], [1, 16]])
                            dst = bass.AP(
                                tensor=s2byT.tensor,
                                offset=s2a.offset + 16 * h,
                                ap=[list(s2a.ap[0]), [HEADS * 16, 16], [1, 16]])
                            evict(dst, src, w=(1, 1, 1))
                        # bounce via DRAM: s2_dram[(72*half+r)*16+t, h, j]
                        s2flat = s2_dram[:]
                        d_dst = bass.AP(tensor=s2flat.tensor,
                                        offset=s2flat.offset + half * HR * 2048,
                                        ap=[[2048, HR], [1, 2048]])
                        s2a = s2byT[:, :, :, :]
                        d_src = bass.AP(tensor=s2byT.tensor, offset=s2a.offset,
                                        ap=[list(s2a.ap[0]), [1, 2048]])
                        nc.sync.dma_start(out=d_dst, in_=d_src)

                    # per-window attention, 3-deep pipeline:
                    # fa(w+2) scores; fb(w+1) softmax+AV; back(w) WO+resid
                    with ExitStack() as hs:
                        psA = hs.enter_context(
                            tc.tile_pool(name="psAT", bufs=2, space="PSUM"))
                        psB = hs.enter_context(
                            tc.tile_pool(name="psBT", bufs=2, space="PSUM"))
                        psC = hs.enter_context(
                            tc.tile_pool(name="psCT", bufs=1, space="PSUM"))

                        def t_fa(w):
                            wg = wlo + w
                            s2w = swp.tile([128, HEADS, 16], BF16, tag="s2w")
                            nc.sync.dma_start(
                                out=s2w[:], in_=s2_dram[128 * wg:128 * (wg + 1)])
                            # emask = mask * exp(scale*s2w), built on Act/Pool
                            # off the critical path
                            eb = swp.tile([128, HEADS, 16], BF16, tag="eb")
                            nc.scalar.activation(out=eb[:], in_=s2w[:],
                                                 func=AF.Exp, scale=SCALE)
                            em = swp.tile([128, HEADS, 128], BF16, tag="em")
                            eba = eb[:, :, :]
                            ebr = bass.AP(tensor=eb.tensor, offset=eba.offset,
                                          ap=[list(eba.ap[0]), [16, HEADS],
                                              [0, 8], [1, 16]])
                            maska = mask[:, :]
                            maskr = bass.AP(tensor=mask.tensor,
                                            offset=maska.offset,
                                            ap=[list(maska.ap[0]), [0, HEADS],
                                                [1, 128]])
                            nc.gpsimd.tensor_tensor(out=em[:], in0=maskr,
                                                    in1=ebr, op=ALU.mult)
                            pS = psA.tile([128, 1024], F32, tag="pS")
                            for h in range(HEADS):
                                nc.tensor.matmul(
                                    pS[:, 128 * h:128 * (h + 1)],
                                    qT[:, h, 128 * w:128 * (w + 1)],
                                    kT[:, h, 128 * w:128 * (w + 1)],
                                    start=True, stop=True)
                            return pS, em

        # stage AB: exp + mask + softmax stats + normalize (Act/DVE/Pool)
                        def t_AB(w, pS, em):
                            aG = swp.tile([128, HEADS, 128], BF16, tag="aG")
                            nc.scalar.activation(out=aG[:], in_=pS[:],
                                                 func=AF.Exp, scale=SCALE)
                            nc.vector.tensor_tensor(out=aG[:], in0=aG[:],
                                                    in1=em[:], op=ALU.mult)
                            aD = swp.tile([128, HEADS, 16], F32, tag="aD")
                            aGa = aG[:, :, :]
                            agv = bass.AP(tensor=aG.tensor, offset=aGa.offset,
                                          ap=[list(aGa.ap[0]), [128, HEADS],
                                              [1, 16], [16, 8]])
                            nc.vector.tensor_reduce(out=aD[:], in_=agv,
                                                    axis=AX.X, op=ALU.add)
                            zt = small.tile([128, HEADS], F32, tag="zt")
                            nc.vector.tensor_reduce(out=zt[:], in_=aD[:],
                                                    axis=AX.X, op=ALU.add)
                            nc.vector.reciprocal(out=zt[:], in_=zt[:])
                            zta = zt[:, :]
                            rzb = bass.AP(tensor=zt.tensor, offset=zta.offset,
                                          ap=[list(zta.ap[0]), [1, HEADS],
                                              [0, 128]])
                            nc.gpsimd.tensor_tensor(out=aG[:], in0=aG[:],
                                                    in1=rzb, op=ALU.mult)
                            if debug and prefix == "t1" and wlo + w == 0:
                                nc.sync.dma_start(out=dbg["aG"][:], in_=aG[:])
                                nc.sync.dma_start(out=dbg["v0"][:],
                                                  in_=v[:, 0, :])
                            rzb2 = bass.AP(tensor=zt.tensor, offset=zta.offset,
                                           ap=[list(zta.ap[0]), [1, HEADS],
                                               [0, 16]])
                            aDn = swp.tile([128, HEADS, 16], BF16, tag="aDn")
                            nc.gpsimd.tensor_tensor(out=aDn[:], in0=aD[:],
                                                    in1=rzb2, op=ALU.mult)
                            return aG, aDn

                        # stage CD: transposes + AV (+rel-V) + oT assembly
                        def t_CD(w, aG, aDn):
                            paT = psB.tile([128, 1024], BF16, tag="ptr")
                            for h in range(HEADS):
                                nc.tensor.transpose(
                                    paT[:, 128 * h:128 * (h + 1)], aG[:, h, :],
                                    identb[:])
                            aTs = swp.tile([128, HEADS, 128], BF16, tag="aTs")
                            nc.scalar.copy(out=aTs[:], in_=paT[:])
                            pdT = psB.tile([128, 1024], BF16, tag="ptr")
                            for h in range(HEADS):
                                nc.tensor.transpose(
                                    pdT[:16, 128 * h:128 * (h + 1)],
                                    aDn[:, h, :], identb[:])
                            aDT = swp.tile([16, HEADS, 128], BF16, tag="aDT")
                            nc.scalar.copy(out=aDT[:], in_=pdT[:16, :])
                            # o1 = v^T A (plain start/stop groups per slot)
                            pO = psA.tile([128, 1024], F32, tag="pS")
                            for h in range(HEADS):
                                nc.tensor.matmul(pO[:DH, 128 * h:128 * (h + 1)],
                                                 v[:, w, DH * h:DH * (h + 1)],
                                                 aTs[:, h, :],
                                                 start=True, stop=True)
                            # o2: disjoint strided cols, own psum, no accum;
                            # one 64-col matmul per frame t (all heads+rows)
                            pR = psC.tile([128, 1024], F32, tag="po2")
                            aDa = aDT[:, :, :]
                            pRa = pR[:, :]
                            # one 32-col matmul per (frame t, head-half hh):
                            # the 4-head span stays inside one psum bank
                            for t in range(T):
                                for hh in range(2):
                                    off = 512 * hh + t
                                    rhs = bass.AP(
                                        tensor=aDT.tensor,
                                        offset=aDa.offset + off,
                                        ap=[list(aDa.ap[0]), [128, 4],
                                            [16, 8]])
                                    ov = bass.AP(
                                        tensor=pR.tensor,
                                        offset=pRa.offset + off,
                                        ap=[[pRa.ap[0][0], DH], [128, 4],
                                            [16, 8]])
                                    nc.tensor.matmul(ov, rvs[:, t, :], rhs,
                                                     start=True, stop=True)
                            oT = swp.tile([DH, HEADS, 128], BF16, tag="oTt")
                            pOa = pO[:, :]
                            src0 = bass.AP(tensor=pO.tensor, offset=pOa.offset,
                                           ap=[[pOa.ap[0][0], DH], [128, HEADS],
                                               [1, 128]])
                            src1 = bass.AP(tensor=pR.tensor, offset=pRa.offset,
                                           ap=[[pRa.ap[0][0], DH], [128, HEADS],
                                               [1, 128]])
                            nc.scalar.copy(out=oT[:, :, :], in_=src0)
                            nc.vector.tensor_tensor(out=oT[:, :, :], in0=src1,
                                                    in1=oT[:, :, :], op=ALU.add)
                            if debug and prefix == "t1" and wlo + w == 0:
                                nc.sync.dma_start(out=dbg["oT0"][:], in_=oT[:])
                            return oT

                        def t_back(w, oT):
                            wg = wlo + w
                            wo_resid(psA, "pS", oT, 0, 128, wo, x_sb[:, wg, :])
                            if debug:
                                nc.sync.dma_start(
                                    out=dbg[dbg_key][:].rearrange(
                                        "r t c -> (r t) c")[128 * wg:128 * (wg + 1), :],
                                    in_=x_sb[:, wg, :])

                        # staged pipeline, emission order chosen so every
                        # engine queue is in ready order (in-order queues):
                        # AB(k+1); CD(k); fa(k+2); back(k)
                        fa_q, ab_q = {}, {}
                        fa_q[0] = t_fa(0)
                        fa_q[1] = t_fa(1)
                        ab_q[0] = t_AB(0, *fa_q.pop(0))
                        v_proj(psC)
                        for k in range(HALFW):
                            if k + 1 < HALFW:
                                ab_q[k + 1] = t_AB(k + 1, *fa_q.pop(k + 1))
                            oT = t_CD(k, *ab_q.pop(k))
                            if k + 2 < HALFW:
                                fa_q[k + 2] = t_fa(k + 2)
                            t_back(k, oT)

        _mark("phaseA")
        temporal("t1", "t1", filler=fill_half)
        _mark("t1")

        # =====================================================================
        # Cross-attention
        # =====================================================================
        with ExitStack() as ph:
            zp = ph.enter_context(tc.tile_pool(name="zpX", bufs=2))
            qp = ph.enter_context(tc.tile_pool(name="qpX", bufs=2))

            for half in range(2):
                wlo = half * HALFW
                zT = zp.tile([128, CHUNKS, HTOK], BF16, tag="zTx")
                qT = qp.tile([DH, HEADS, HTOK], BF16, tag="qx")
                with ExitStack() as hs:
                    psz = hs.enter_context(tc.tile_pool(name="pszX", bufs=2,
                                                        space="PSUM"))
                    ln_fm(psz, zp, lambda w: x_sb[:, wlo + w, :], zT, HALFW)
                    proj_fm(psz, zT, wqx, qT, HTOK)
                with ExitStack() as hs:
                    pss = hs.enter_context(tc.tile_pool(name="pssX", bufs=2,
                                                        space="PSUM"))
                    psB = hs.enter_context(tc.tile_pool(name="psBX", bufs=2,
                                                        space="PSUM"))
                    eS = qp.tile([77, HEADS, HTOK], BF16, tag="eSx")

                    def escores(o):
                        # eS chunk [o, o+n): only needed by windows >= o//128,
                        # so later chunks are emitted after the window loop
                        # starts — their Act-bound exps overlap window PE work
                        n = min(512, HTOK - o)
                        for h in range(HEADS):
                            ps = pss.tile([77, 512], F32, tag="psx")
                            nc.tensor.matmul(ps[:, 0:n], kctxT[:, h, :],
                                             qT[:, h, o:o + n],
                                             start=True, stop=True)
                            nc.scalar.activation(out=eS[:, h, o:o + n],
                                                 in_=ps[:, 0:n],
                                                 func=AF.Exp, scale=SCALE)

                    def x_fa(w):
                        oX = psB.tile([128, 1024], F32, tag="oX")
                        for h in range(HEADS):
                            nc.tensor.matmul(oX[:, 128 * h:128 * h + DH + 1],
                                             eS[:, h, 128 * w:128 * (w + 1)],
                                             v1x[:, h, :],
                                             start=True, stop=True)
                        return oX

                    def x_fb(w, oX):
                        rz = small.tile([128, HEADS], F32, tag="rzx")
                        oXa = oX[:, :]
                        zv = bass.AP(tensor=oX.tensor, offset=oXa.offset + DH,
                                     ap=[list(oXa.ap[0]), [128, HEADS]])
                        nc.vector.reciprocal(out=rz[:], in_=zv)
                        oN = zp.tile([128, HEADS, DH], BF16, tag="oNx")
                        src = bass.AP(tensor=oX.tensor, offset=oXa.offset,
                                      ap=[list(oXa.ap[0]), [128, HEADS],
                                          [1, DH]])
                        rza = rz[:, :]
                        rzb = bass.AP(tensor=rz.tensor, offset=rza.offset,
                                      ap=[list(rza.ap[0]), [1, HEADS], [0, DH]])
                        nc.vector.tensor_tensor(out=oN[:], in0=src, in1=rzb,
                                                op=ALU.mult)
                        pt = psB.tile([DH, HEADS, 128], BF16, tag="ptx")
                        for h in range(HEADS):
                            nc.tensor.transpose(pt[:, h, :], oN[:, h, :],
                                                identb[:])
                        oTx = zp.tile([DH, HEADS, 128], BF16, tag="oTx")
                        evict(oTx[:], pt[:], w=(2, 1, 0))
                        return oTx

                    def x_back(w, oTx):
                        wg = wlo + w
                        wo_resid(psB, "oX", oTx, 0, 128, wox, x_sb[:, wg, :])
                        if debug:
                            nc.sync.dma_start(
                                out=dbg["x2"][:].rearrange(
                                    "r t c -> (r t) c")[128 * wg:128 * (wg + 1), :],
                                in_=x_sb[:, wg, :])

                    # staged: fb(k+1) before fa(k+2) before back(k), so each
                    # engine queue stays in ready order; eS score chunks for
                    # later windows are emitted mid-loop
                    escores(0)
                    fa_q, fb_q = {}, {}
                    fa_q[0] = x_fa(0)
                    fa_q[1] = x_fa(1)
                    fb_q[0] = x_fb(0, fa_q.pop(0))
                    escores(512)
                    for k in range(HALFW):
                        if k + 1 < HALFW:
                            fb_q[k + 1] = x_fb(k + 1, fa_q.pop(k + 1))
                        if k + 2 < HALFW:
                            fa_q[k + 2] = x_fa(k + 2)
                        x_back(k, fb_q.pop(k))
                        if k == 2:
                            escores(1024)

        _mark("cross")
        temporal("t2", "t2")
        _mark("t2")

        # =====================================================================
        # GEGLU FFN per window. ff_w1 cols host-permuted into rounds of
        # (4 a-chunks, 4 gate-chunks); a-chunk order preserved for ff_w2.
        # =====================================================================
        with ExitStack() as ph:
            wp = ph.enter_context(tc.tile_pool(name="wpF", bufs=1))
            zp = ph.enter_context(tc.tile_pool(name="zpF", bufs=2))
            hp = ph.enter_context(tc.tile_pool(name="hpF", bufs=2))
            psp = ph.enter_context(tc.tile_pool(name="psF", bufs=2, space="PSUM"))
            psx = ph.enter_context(tc.tile_pool(name="psxF", bufs=1, space="PSUM"))
            psh = ph.enter_context(tc.tile_pool(name="pshF", bufs=2, space="PSUM"))

            # w1 loaded per 1024-col round so the first super-window's
            # matmuls start after ~1/5 of the 6.5MB instead of all of it
            w1 = wp.tile([128, CHUNKS, 2 * FFI], BF16, tag="w1")
            for r in range(5):
                nc.sync.dma_start(
                    out=w1[:, :, 1024 * r:1024 * (r + 1)],
                    in_=wts["ff_w1"][:, 1024 * r:1024 * (r + 1)].rearrange(
                        "(a p) n -> p a n", p=128))
            w2 = wp.tile([128, NG2, C], BF16, tag="w2")
            for ci0 in range(0, NG2, 10):
                nc.sync.dma_start(
                    out=w2[:, ci0:ci0 + 10, :],
                    in_=wts["ff_w2"][128 * ci0:128 * (ci0 + 10), :].rearrange(
                        "(a p) n -> p a n", p=128))

            stats = ln_stats(wp, lambda w: x_sb[:, w, :], NWIN, "lnF")
            for sw in range(5):
                w0 = 4 * sw
                nw = 4 if sw < 4 else 2
                ntok = 128 * nw
                zT = zp.tile([128, CHUNKS, 512], BF16, tag="zTf")
                ln_fm(psp, zp, lambda i: x_sb[:, w0 + i, :], zT, nw,
                      stats=stats, w0=w0)
                uT = hp.tile([128, NG2, 512], BF16, tag="uT")
                for r in range(5):
                    for p in range(4):
                        ph_ = psh.tile([128, 2, 512], F32, tag="ph")
                        for j, co in ((0, p), (1, 4 + p)):
                            gcol = 1024 * r + 128 * co
                            for ci in range(CHUNKS):
                                nc.tensor.matmul(ph_[:, j, 0:ntok],
                                                 w1[:, ci, gcol:gcol + 128],
                                                 zT[:, ci, 0:ntok],
                                                 start=(ci == 0),
                                                 stop=(ci == CHUNKS - 1))
                        gl = hp.tile([128, 512], BF16, tag="gelu")
                        nc.scalar.activation(out=gl[:, 0:ntok],
                                             in_=ph_[:, 1, 0:ntok],
                                             func=AF.Gelu)
                        nc.vector.tensor_tensor(out=uT[:, 4 * r + p, 0:ntok],
                                                in0=ph_[:, 0, 0:ntok],
                                                in1=gl[:, 0:ntok],
                                                op=ALU.mult)
                for i in range(nw):
                    w = w0 + i
                    px = psx.tile([128, 1024], F32, tag="px")
                    for o, n in ((0, 512), (512, 128)):
                        for ci in range(NG2):
                            nc.tensor.matmul(px[:, o:o + n],
                                             uT[:, ci, 128 * i:128 * (i + 1)],
                                             w2[:, ci, o:o + n],
                                             start=(ci == 0), stop=(ci == NG2 - 1))
                    xout = zp.tile([128, C], F32, tag="xout")
                    nc.vector.scalar_tensor_tensor(
                        out=xout[:], in0=px[:, 0:C], scalar=1.0,
                        in1=x_sb[:, w, :], op0=ALU.mult, op1=ALU.add)
                    nc.sync.dma_start(
                        out=out_final[:].rearrange(
                            "r t c -> (r t) c")[128 * w:128 * (w + 1), :],
                        in_=xout[:])

    _mark("ffn")
    bass.BassTensorEngine.matmul = _omm
    if not nc.is_finalized():
        nc.finalize()
    return nc


# ----------------------------------------------------------------------------
# host side
# ----------------------------------------------------------------------------

def _bf(a):
    return np.asarray(a, dtype=ml_dtypes.bfloat16)


def prepare_inputs(inputs):
    f = {k: np.asarray(v, dtype=np.float32) for k, v in inputs.items()}
    shared = {}

    def fold(g, b, wname):
        wf = f[wname]
        bias = f[b] @ wf
        assert np.abs(bias).max() < 1e-6, f"nonzero folded bias for {wname}"
        return f[g][:, None] * wf

    for k in ("a1_bo", "a2_bo", "t1_bo", "t2_bo", "ff_b1", "ff_b2"):
        assert np.abs(f[k]).max() < 1e-6, f"nonzero bias {k} unsupported"

    for p, gk, bk_ in (("a1", "g1", "b1"), ("t1", "g4", "b4"),
                       ("t2", "g5", "b5")):
        for kind in ("wq", "wk", "wv"):
            shared[f"{p}_{kind}"] = _bf(fold(gk, bk_, f"{p}_{kind}"))
    shared["a2_wq"] = _bf(fold("g2", "b2", "a2_wq"))
    shared["a2_wk"] = _bf(f["a2_wk"])
    shared["a2_wv"] = _bf(f["a2_wv"])
    for p in ("a1", "a2", "t1", "t2"):
        shared[f"{p}_wo"] = _bf(
            f[f"{p}_wo"].reshape(HEADS, DH, C).transpose(1, 0, 2))
    for p in ("t1", "t2"):
        shared[f"{p}_rkT"] = _bf(f[f"{p}_rk"].T)
        rv = f[f"{p}_rv"]
        rvs = np.zeros((16, T, DH), np.float32)
        for t in range(T):
            for j in range(16):
                rvs[j, t] = rv[j - t + MAXREL]
        shared[f"{p}_rvs"] = _bf(rvs)
    w1f = fold("g3", "b3", "ff_w1")
    a_, g_ = w1f[:, :FFI], w1f[:, FFI:]
    cols = []
    for r in range(5):
        cols.append(a_[:, 512 * r:512 * (r + 1)])
        cols.append(g_[:, 512 * r:512 * (r + 1)])
    shared["ff_w1"] = _bf(np.concatenate(cols, axis=1))
    shared["ff_w2"] = _bf(f["ff_w2"])
    m = np.zeros((128, 128), np.float32)
    for g in range(8):
        m[16 * g:16 * (g + 1), 16 * g:16 * (g + 1)] = 1.0
    shared["bd_mask"] = _bf(m)

    x = f["x"]
    ctx = f["context"]
    in_maps = []
    for core in range(NCORES):
        im = dict(shared)
        xs = np.empty((NG, SEQ, C), np.float32)
        for g in range(NG):
            bt = core + 8 * g
            b, t = bt // T, bt % T
            xs[g] = x[b, :, t].reshape(C, SEQ).T
        im["xs_in"] = _bf(xs)
        im["ctxT"] = _bf(ctx[core // 4].T.copy())
        in_maps.append(im)
    return in_maps


_PROGRAM_CACHE = {}


def run(inputs, debug=False, trace=False):
    key = "dbg" if debug else "plain"
    if key not in _PROGRAM_CACHE:
        _PROGRAM_CACHE[key] = build_program(debug=debug)
    nc = _PROGRAM_CACHE[key]
    in_maps = prepare_inputs(inputs)
    from concourse.bass_utils import run_bass_kernel_spmd
    res = run_bass_kernel_spmd(nc, in_maps, list(range(NCORES)), trace=trace)
    outs = res.results
    full = np.empty((B * H * W, T, C), np.float32)
    for core in range(NCORES):
        full[NR * core:NR * (core + 1)] = outs[core]["out"]
    y = full.reshape(B, H, W, T, C).transpose(0, 4, 3, 1, 2)
    return y, res, outs


def kernel(**inputs):
    y, _, _ = run(inputs)
    return y.astype(np.float32)



# revision 123
# speedup vs baseline: 1.0015x; 1.0005x over previous
"""Trainium2 Bass kernel for BasicTransformerBlockST (spatial/temporal block).

Sharding over 8 NeuronCores (same as baseline):
  Phase A (spatial self-attn): data-parallel over (b,t): core i owns the 4
  groups bt = i + 8g. An 8-way AllToAll (split in two, overlapped with phase
  A compute) reshards to (b,h,w)-parallel: core j owns rows
  (b=j//4, hw in [144*(j%4), 144*(j%4+1))), tokens r-major (token = r*16+t).
  t1 / cross-attn / t2 / FFN run on that shard with the residual stream
  resident in SBUF (no DRAM bounces).

Optimized for the TimelineSim cost model: batched big instructions, S^T
softmax formulation (no attention-matrix transposes or renormalize in phase
A / cross), z via ones-column fused into AV, evictions spread across
DVE/Act/Pool, PSUM tag sharing for double buffering.

Scheduling notes (engines execute their queues strictly in order, so
emission order is the schedule):
 - residual crosses the AllToAll in bf16; each slot is split into row-half
   collectives in separate DRAM tensors so t1 half-0 starts during the
   second collective
 - temporal/cross window loops are staged pipelines emitted in per-engine
   ready order (AB(k+1); CD(k); fa(k+2); back(k))
 - V projection is emitted after the window-loop prologue so its PE work
   fills the softmax pipeline-fill bubble
 - LN stats are batched per phase (one Sqrt act-table episode each)
 - a single matmul's PSUM output span must stay inside one 2KB bank
   (the o2 rel-V matmuls are split per head-half for this); accumulating
   with start=False onto a region written by a different matmul shape
   produces wrong results on this stack - keep o1/o2 in separate psum
"""

import sys

sys.path.insert(0, "/opt/trn_rl_repo")

import numpy as np
import ml_dtypes

import concourse.bass as bass
import concourse.bacc as bacc
import concourse.mybir as mybir
import concourse.tile as tile
from concourse.masks import make_identity

F32 = mybir.dt.float32
BF16 = mybir.dt.bfloat16
AF = mybir.ActivationFunctionType
ALU = mybir.AluOpType
AX = mybir.AxisListType

B, C, T, H, W = 2, 640, 16, 24, 24
HEADS, DH = 8, 80
CTXD = 1024
MAXREL = 16
NREL = 2 * MAXREL + 1          # 33
FFI = 4 * C                    # 2560
INNER = HEADS * DH             # 640
SCALE = DH ** -0.5
EPS = 1e-5

NCORES = 8
NG = 4                         # spatial groups per core
SEQ = H * W                    # 576
NR = (B * H * W) // NCORES     # 144 rows per core
TOK = NR * T                   # 2304 tokens per core
NWIN = TOK // 128              # 18
CHUNKS = C // 128              # 5
CTXCH = CTXD // 128            # 8
HALFW = NWIN // 2              # 9 windows per temporal half
HR = NR // 2                   # 72 rows per half
HTOK = 128 * HALFW             # 1152 tokens per half
NG2 = FFI // 128               # 20 ffn chunks

# token chunks of a 576-token spatial group
QSP = [(0, 128), (128, 128), (256, 128), (384, 128), (512, 64)]


PHASE_MARKS = []


def build_program(debug=False):
    nc = bacc.Bacc(None, target_bir_lowering=False)

    # instrument PE-instruction counts at phase boundaries (analysis only)
    PHASE_MARKS.clear()
    _mmcnt = [0]
    _omm = bass.BassTensorEngine.matmul

    def _cmm(self, *a, **k):
        _mmcnt[0] += 1
        return _omm(self, *a, **k)

    bass.BassTensorEngine.matmul = _cmm

    def _mark(name):
        PHASE_MARKS.append((name, _mmcnt[0]))

    xs_in = nc.dram_tensor("xs_in", [NG, SEQ, C], BF16, kind="ExternalInput")
    ctxT_in = nc.dram_tensor("ctxT", [CTXD, 77], BF16, kind="ExternalInput")

    def win(name, shape, dt=BF16):
        return nc.dram_tensor(name, shape, dt, kind="ExternalInput")

    wts = {}
    for p in ("a1", "a2", "t1", "t2"):
        cin = CTXD if p == "a2" else C
        wts[f"{p}_wq"] = win(f"{p}_wq", [C, INNER])
        wts[f"{p}_wk"] = win(f"{p}_wk", [cin, INNER])
        wts[f"{p}_wv"] = win(f"{p}_wv", [cin, INNER])
        wts[f"{p}_wo"] = win(f"{p}_wo", [DH, HEADS, C])
    for p in ("t1", "t2"):
        wts[f"{p}_rkT"] = win(f"{p}_rkT", [DH, NREL])
        wts[f"{p}_rvs"] = win(f"{p}_rvs", [16, T, DH])  # rvs[j,t,d]=rv[j-t+16,d]
    wts["ff_w1"] = win("ff_w1", [C, 2 * FFI])  # host-permuted cols (4a,4g)
    wts["ff_w2"] = win("ff_w2", [FFI, C])
    bd_mask = win("bd_mask", [128, 128], BF16)

    out_final = nc.dram_tensor("out", [NR, T, C], F32, kind="ExternalOutput")
    dbg = {}
    if debug:
        dbg["a"] = nc.dram_tensor("dbg_a", [NG, SEQ, C], BF16,
                                  kind="ExternalOutput")
        for nm in ("t1", "x2", "t2"):
            dbg[nm] = nc.dram_tensor(f"dbg_{nm}", [NR, T, C], F32,
                                     kind="ExternalOutput")
        dbg["aG"] = nc.dram_tensor("dbg_aG", [128, HEADS, 128], BF16,
                                   kind="ExternalOutput")
        dbg["v0"] = nc.dram_tensor("dbg_v0", [128, C], BF16,
                                   kind="ExternalOutput")
        dbg["q0"] = nc.dram_tensor("dbg_q0", [DH, HEADS, 128], BF16,
                                   kind="ExternalOutput")
        dbg["oT0"] = nc.dram_tensor("dbg_oT0", [DH, HEADS, 128], BF16,
                                    kind="ExternalOutput")

    # slot-major a2a: slot s holds frames t = i + 8*s from src core i.
    # One tensor per row half (rh) so the temporal phase's half-0 fill
    # only depends on the rh=0 collectives; slot is the leading dim.
    a2a_in = [nc.dram_tensor(f"a2a_in{r}", [2, NCORES, HR, C], BF16)
              for r in range(2)]
    a2a_out = [nc.dram_tensor(f"a2a_out{r}", [2, NCORES, HR, C], BF16)
               for r in range(2)]
    s2_dram = nc.dram_tensor("s2_dram", [TOK, HEADS, 16], BF16)
    groups = [[0, 1, 2, 3, 4, 5, 6, 7]]

    from contextlib import ExitStack

    with tile.TileContext(nc) as tc, ExitStack() as top:
        const = top.enter_context(tc.tile_pool(name="const", bufs=1))
        identb = const.tile([128, 128], BF16)
        make_identity(nc, identb)
        eps_t = const.tile([128, 1], F32)
        nc.vector.memset(eps_t[:], EPS)
        mask = const.tile([128, 128], BF16)
        nc.sync.dma_start(out=mask[:], in_=bd_mask[:, :])
        small = top.enter_context(tc.tile_pool(name="small", bufs=6))
        resp = top.enter_context(tc.tile_pool(name="resp", bufs=1))
        x_sb = resp.tile([128, NWIN, C], F32, tag="x_sb")

        ev_state = [0]

        def evict(out, in_, w=(1, 1, 1)):
            """psum->sbuf copy via rotating engines; w=(dve, act, act2).
            GPSIMD cannot access PSUM, so only DVE/Act here."""
            seq = [nc.vector] * w[0] + [nc.scalar] * (w[1] + w[2])
            eng = seq[ev_state[0] % len(seq)]
            ev_state[0] += 1
            if eng is nc.scalar:
                eng.copy(out=out, in_=in_)
            else:
                eng.tensor_copy(out=out, in_=in_)

        def ln_stats(sp, xfn, nw, tag):
            """Batched LN stats: one Sqrt activation for all nw windows so
            the Act engine swaps function tables once per phase, not per
            window. Returns (mv [128,nw,2], rstd [128,nw])."""
            mv = sp.tile([128, nw, 2], F32, tag=tag + "_mv")
            for w in range(nw):
                x = xfn(w)
                st = small.tile([128, 2, 6], F32, tag="bnst")
                nc.vector.bn_stats(out=st[:, 0, :], in_=x[:, 0:512])
                nc.vector.bn_stats(out=st[:, 1, :], in_=x[:, 512:640])
                nc.vector.bn_aggr(out=mv[:, w, :], in_=st[:])
            rstd = sp.tile([128, nw], F32, tag=tag + "_rs")
            mva = mv[:, :, :]
            var = bass.AP(tensor=mv.tensor, offset=mva.offset + 1,
                          ap=[list(mva.ap[0]), [2, nw]])
            nc.scalar.activation(out=rstd[:], in_=var, func=AF.Sqrt,
                                 bias=eps_t[:], scale=1.0)
            nc.vector.reciprocal(out=rstd[:], in_=rstd[:])
            return mv, rstd

        def ln_fm(psp, zp, xfn, zT, nw, stats=None, w0=0, zs_eng=None):
            """LayerNorm (g/b folded into weights) + transpose into
            feature-major zT[:, ci, 128*w : 128*w+128] bf16. zs_eng=Pool for
            the startup group only: its apply would otherwise queue on DVE
            behind the serial bn_stats."""
            zT_a = zT[:, :, :]
            ntok = zT_a.ap[1][0]
            if stats is None:
                stats = ln_stats(zp, xfn, nw, "lnf")
            mv, rstd = stats
            for w in range(nw):
                x = xfn(w)
                zs = zp.tile([128, C], BF16, tag="zs")
                (zs_eng or nc.vector).tensor_scalar(
                    out=zs[:], in0=x, scalar1=mv[:, w0 + w, 0:1],
                    scalar2=rstd[:, w0 + w:w0 + w + 1],
                    op0=ALU.subtract, op1=ALU.mult)
                pz = psp.tile([128, CHUNKS, 128], BF16, tag="pz")
                for c in range(CHUNKS):
                    nc.tensor.transpose(pz[:, c, :], zs[:, 128 * c:128 * (c + 1)],
                                        identb[:])
                dst = bass.AP(tensor=zT.tensor,
                              offset=zT_a.offset + 128 * w,
                              ap=[list(zT_a.ap[0]), [ntok, CHUNKS], [1, 128]])
                evict(dst, pz[:, :, :], w=(2, 1, 1))

        def load_w_cin(wp, name, cin):
            t = wp.tile([128, cin // 128, wts[name].shape[-1]], BF16,
                        tag="w_" + name)
            nc.sync.dma_start(out=t[:],
                              in_=wts[name][:].rearrange("(a p) n -> p a n", p=128))
            return t

        def load_wo(wp, name):
            t = wp.tile([DH, HEADS, C], BF16, tag="w_" + name)
            nc.sync.dma_start(out=t[:], in_=wts[name][:])
            return t

        def proj_fm(psp, zT, w_sb, qT, ntok):
            """feature-major projection qT[80, h, ntok] (bf16).
            PSUM allocations cap at 4KB, so one 1-bank tile per 512-split."""
            for h in range(HEADS):
                for o in range(0, ntok, 512):
                    n = min(512, ntok - o)
                    pq = psp.tile([128, 512], F32, tag="pA")
                    for ci in range(CHUNKS):
                        nc.tensor.matmul(pq[:DH, 0:n],
                                         w_sb[:, ci, DH * h:DH * (h + 1)],
                                         zT[:, ci, o:o + n],
                                         start=(ci == 0), stop=(ci == CHUNKS - 1))
                    evict(qT[:, h, o:o + n], pq[:DH, 0:n], w=(2, 2, 1))

        def wo_resid(psp, tag, oT, qoff, ntok, wo_sb, resid_ap):
            """WO projection (by-head lhsT oT[:, h, qoff:qoff+ntok]) +
            residual add into resid_ap [ntok, C]."""
            pw = psp.tile([128, 1024], F32, tag=tag)
            for o, n in ((0, 512), (512, 128)):
                for h in range(HEADS):
                    nc.tensor.matmul(pw[:ntok, o:o + n],
                                     oT[:, h, qoff:qoff + ntok],
                                     wo_sb[:, h, o:o + n],
                                     start=(h == 0), stop=(h == HEADS - 1))
            nc.vector.scalar_tensor_tensor(
                out=resid_ap, in0=pw[:ntok, 0:C], scalar=1.0, in1=resid_ap,
                op0=ALU.mult, op1=ALU.add)

        # =====================================================================
        # PHASE A: spatial self-attention per (b,t) group; order 0,2,1,3 so
        # each a2a slot's collective fires after two groups.
        # =====================================================================
        with ExitStack() as ph:
            xap = ph.enter_context(tc.tile_pool(name="xapA", bufs=1))
            wp = ph.enter_context(tc.tile_pool(name="wpA", bufs=1))
            zp = ph.enter_context(tc.tile_pool(name="zpA", bufs=2))
            qp = ph.enter_context(tc.tile_pool(name="qpA", bufs=2))
            ap_ = ph.enter_context(tc.tile_pool(name="apA", bufs=2))
            psp = ph.enter_context(tc.tile_pool(name="psA", bufs=3, space="PSUM"))
            pso = ph.enter_context(tc.tile_pool(name="psoA", bufs=2, space="PSUM"))

            # all 4 groups' inputs DMA'd up front (before the weight loads in
            # the DMA queue) + one batched LN-stats pass: a single Sqrt table
            # episode for the whole phase instead of one per group
            xall = xap.tile([128, NG, CHUNKS, C], BF16, tag="xall")

            def xdma(g):
                nc.sync.dma_start(out=xall[:, g, 0:4, :],
                                  in_=xs_in[g, 0:512, :].rearrange(
                                      "(a p) c -> p a c", p=128))
                nc.sync.dma_start(out=xall[:64, g, 4, :],
                                  in_=xs_in[g, 512:576, :])

            # two stats batches: group 0's normalize starts after 10
            # windows' stats instead of all 20
            xdma(0)
            xdma(2)
            statsA1 = ln_stats(wp, lambda k: xall[:, (0, 2)[k // 5],
                                                  k % 5, :],
                               2 * CHUNKS, "lnA1")
            xdma(1)
            xdma(3)
            statsA2 = ln_stats(wp, lambda k: xall[:, (1, 3)[k // 5],
                                                  k % 5, :],
                               2 * CHUNKS, "lnA2")
            smap = {0: (statsA1, 0), 2: (statsA1, 5),
                    1: (statsA2, 0), 3: (statsA2, 5)}

            wq = load_w_cin(wp, "a1_wq", C)
            wk = load_w_cin(wp, "a1_wk", C)
            wv = load_w_cin(wp, "a1_wv", C)
            wo = load_wo(wp, "a1_wo")

            for g in (0, 2, 1, 3):
                zT = zp.tile([128, CHUNKS, 640], BF16, tag="zTa")
                ln_fm(pso, zp, lambda w, g=g: xall[:, g, w, :], zT, 5,
                      stats=smap[g][0], w0=smap[g][1],
                      zs_eng=nc.gpsimd if g == 0 else None)

                qT = qp.tile([DH, HEADS, SEQ], BF16, tag="qa")
                kT = qp.tile([DH, HEADS, SEQ], BF16, tag="ka")
                proj_fm(psp, zT, wq, qT, SEQ)
                proj_fm(psp, zT, wk, kT, SEQ)

                # v token-major with ones column per head (memset 1.0 first;
                # the projection evictions overwrite all but the ones column)
                v1 = qp.tile([128, CHUNKS, HEADS, DH + 1], BF16, tag="va")
                nc.gpsimd.memset(v1[:], 1.0)
                for (w, (o_, np_)) in enumerate(QSP):
                    pv = psp.tile([128, 1024], F32, tag="pA")
                    for o, n in ((0, 512), (512, 128)):
                        for ci in range(CHUNKS):
                            nc.tensor.matmul(pv[:np_, o:o + n],
                                             zT[:, ci, o_:o_ + np_],
                                             wv[:, ci, o:o + n],
                                             start=(ci == 0), stop=(ci == CHUNKS - 1))
                    v1a = v1[:, :, :, :]
                    dst = bass.AP(tensor=v1.tensor,
                                  offset=v1a.offset + w * HEADS * (DH + 1),
                                  ap=[[v1a.ap[0][0], np_], [DH + 1, HEADS],
                                      [1, DH]])
                    evict(dst, pv[:np_, 0:C], w=(2, 1, 1))

                oT = ap_.tile([DH, HEADS, SEQ], BF16, tag="oa")

                def a_front(h):
                    """scores exp(S^T) for head h"""
                    eS = ap_.tile([128, CHUNKS, SEQ], BF16, tag="eS")
                    for (kc, (ko, kp)) in enumerate(QSP):
                        ps = psp.tile([128, 1024], F32, tag="pA")
                        for o, n in ((0, 512), (512, 64)):
                            nc.tensor.matmul(ps[:kp, o:o + n],
                                             kT[:, h, ko:ko + kp],
                                             qT[:, h, o:o + n],
                                             start=True, stop=True)
                        nc.scalar.activation(out=eS[:kp, kc, 0:SEQ],
                                             in_=ps[:kp, 0:SEQ],
                                             func=AF.Exp, scale=SCALE)
                    return eS

                def a_back(h, eS):
                    # AV + z via ones column: oA[q, 80] = z
                    oA = pso.tile([128, CHUNKS, 96], F32, tag="pz")
                    for (qc, (qo, qp_)) in enumerate(QSP):
                        for (kc, (ko, kp)) in enumerate(QSP):
                            nc.tensor.matmul(oA[:qp_, qc, 0:DH + 1],
                                             eS[:kp, kc, qo:qo + qp_],
                                             v1[:kp, kc, h, :],
                                             start=(kc == 0), stop=(kc == 4))
                    rz = small.tile([128, CHUNKS], F32, tag="rz")
                    oAa = oA[:, :, :]
                    zv = bass.AP(tensor=oA.tensor, offset=oAa.offset + DH,
                                 ap=[list(oAa.ap[0]), [96, CHUNKS]])
                    nc.vector.reciprocal(out=rz[:], in_=zv)
                    oN = ap_.tile([128, CHUNKS, DH], BF16, tag="oN")
                    src = bass.AP(tensor=oA.tensor, offset=oAa.offset,
                                  ap=[list(oAa.ap[0]), [96, CHUNKS], [1, DH]])
                    rza = rz[:, :]
                    rzb = bass.AP(tensor=rz.tensor, offset=rza.offset,
                                  ap=[list(rza.ap[0]), [1, CHUNKS], [0, DH]])
                    nc.vector.tensor_tensor(out=oN[:], in0=src, in1=rzb,
                                            op=ALU.mult)
                    pt = pso.tile([DH, CHUNKS, 128], BF16, tag="pz")
                    for (qc, (qo, qp_)) in enumerate(QSP):
                        nc.tensor.transpose(pt[:, qc, 0:qp_], oN[:qp_, qc, :],
                                            identb[:qp_, :qp_])
                    pta = pt[:, :, :]
                    src = bass.AP(tensor=pt.tensor, offset=pta.offset,
                                  ap=[list(pta.ap[0]), [128, 4], [1, 128]])
                    evict(oT[:, h, 0:512], src, w=(2, 1, 1))
                    evict(oT[:, h, 512:576], pt[:, 4, 0:64], w=(2, 1, 1))

                # software-pipeline heads: scores(h+1) before AV/norm(h) so
                # the PE never waits on head h's exp chain
                prev_eS = None
                for h in range(HEADS):
                    eS = a_front(h)
                    if prev_eS is not None:
                        a_back(h - 1, prev_eS)
                    prev_eS = eS
                a_back(HEADS - 1, prev_eS)

                # WO + residual (in place on xg), cast to bf16 for the
                # collective, scatter to a2a_in
                b_, tslot = g // 2, g % 2
                for (qc, (qo, qp_)) in enumerate(QSP):
                    xq = xall[:qp_, g, qc, :]
                    wo_resid(psp, "pA", oT, qo, qp_, wo, xq)
                    q0, q1 = qo // HR, (qo + qp_ - 1) // HR
                    for q in range(q0, q1 + 1):
                        lo, hi = max(qo, HR * q), min(qo + qp_, HR * (q + 1))
                        nc.sync.dma_start(
                            out=a2a_in[q % 2][tslot, 4 * b_ + q // 2,
                                              lo - HR * q:hi - HR * q, :],
                            in_=xall[lo - qo:hi - qo, g, qc, :])
                    if debug:
                        nc.sync.dma_start(out=dbg["a"][g, qo:qo + qp_, :],
                                          in_=xall[:qp_, g, qc, :])
                if g == 2:
                    for r in range(2):
                        nc.gpsimd.collective_compute(
                            "AllToAll", ALU.bypass, replica_groups=groups,
                            ins=[a2a_in[r][0]], outs=[a2a_out[r][0]])
            for r in range(2):
                nc.gpsimd.collective_compute(
                    "AllToAll", ALU.bypass, replica_groups=groups,
                    ins=[a2a_in[r][1]], outs=[a2a_out[r][1]])

        # cross-attention KV setup hoisted here: it has no dependency on
        # the AllToAll, so PE/DMA work lands inside the collective gap
        xkv = top.enter_context(tc.tile_pool(name="xkv", bufs=1))
        wqx = load_w_cin(xkv, "a2_wq", C)
        wox = load_wo(xkv, "a2_wo")
        with ExitStack() as hs:
            kvp = hs.enter_context(tc.tile_pool(name="kvpX", bufs=1))
            psk = hs.enter_context(tc.tile_pool(name="pskX", bufs=2,
                                                space="PSUM"))
            wkc = load_w_cin(kvp, "a2_wk", CTXD)
            wvc = load_w_cin(kvp, "a2_wv", CTXD)
            ctx_sb = kvp.tile([128, CTXCH, 77], BF16, tag="ctx")
            nc.sync.dma_start(out=ctx_sb[:],
                              in_=ctxT_in[:].rearrange("(a p) m -> p a m",
                                                       p=128))
            kctxT = xkv.tile([DH, HEADS, 77], BF16, tag="kctx")
            pk = psk.tile([DH, HEADS, 128], F32, tag="pk")
            for h in range(HEADS):
                for ci in range(CTXCH):
                    nc.tensor.matmul(pk[:, h, 0:77],
                                     wkc[:, ci, DH * h:DH * (h + 1)],
                                     ctx_sb[:, ci, :],
                                     start=(ci == 0), stop=(ci == CTXCH - 1))
            pka = pk[:, :, :]
            src = bass.AP(tensor=pk.tensor, offset=pka.offset,
                          ap=[list(pka.ap[0]), [128, HEADS], [1, 77]])
            evict(kctxT[:, :, :], src, w=(1, 1, 1))
            v1x = xkv.tile([77, HEADS, DH + 1], BF16, tag="vctx")
            nc.gpsimd.memset(v1x[:], 1.0)
            pv = psk.tile([77, 1024], F32, tag="pvx")
            for o, n in ((0, 512), (512, 128)):
                for ci in range(CTXCH):
                    nc.tensor.matmul(pv[:, o:o + n], ctx_sb[:, ci, :],
                                     wvc[:, ci, o:o + n],
                                     start=(ci == 0), stop=(ci == CTXCH - 1))
            v1a = v1x[:, :, :]
            dst = bass.AP(tensor=v1x.tensor, offset=v1a.offset,
                          ap=[list(v1a.ap[0]), [DH + 1, HEADS], [1, DH]])
            evict(dst, pv[:, 0:C], w=(1, 1, 1))

        # fill x_sb windows from a2a_out: partition p=16r'+t, t=i+8s;
        # bf16 staging + per-window upcast back to the f32 residual.
        # Done per temporal half (scoped pool) so half-0's pipeline never
        # waits on the half-1 collective.
        def fill_half(half, pool):
            xst = pool.tile([128, HALFW, C], BF16, tag="xst")
            base = a2a_out[half][:]
            for i in range(HALFW):
                src = bass.AP(tensor=base.tensor,
                              offset=base.offset + 8 * i * C,
                              ap=[[C, 8], [NCORES * HR * C, 2],
                                  [HR * C, 8], [1, C]])
                nc.sync.dma_start(out=xst[:, i, :], in_=src)
                w = half * HALFW + i
                if i % 3 == 0:
                    nc.vector.tensor_copy(out=x_sb[:, w, :],
                                          in_=xst[:, i, :])
                elif i % 3 == 1:
                    nc.scalar.copy(out=x_sb[:, w, :], in_=xst[:, i, :])
                else:
                    nc.gpsimd.tensor_copy(out=x_sb[:, w, :],
                                          in_=xst[:, i, :])

        # =====================================================================
        # Temporal attention (t1 / t2), per half
        # =====================================================================
        def temporal(prefix, dbg_key, filler=None):
            with ExitStack() as ph:
                wp = ph.enter_context(tc.tile_pool(name="wpT", bufs=1))
                zp = ph.enter_context(tc.tile_pool(name="zpT", bufs=2))
                ztp = ph.enter_context(tc.tile_pool(name="ztpT", bufs=1))
                qp = ph.enter_context(tc.tile_pool(name="qpT", bufs=1))
                swp = ph.enter_context(tc.tile_pool(name="swpT", bufs=2))
                sp2 = ph.enter_context(tc.tile_pool(name="sp2T", bufs=2))

                wq = load_w_cin(wp, f"{prefix}_wq", C)
                wk = load_w_cin(wp, f"{prefix}_wk", C)
                wv = load_w_cin(wp, f"{prefix}_wv", C)
                wo = load_wo(wp, f"{prefix}_wo")
                rkT = wp.tile([DH, NREL], BF16, tag="rkT")
                nc.sync.dma_start(out=rkT[:], in_=wts[f"{prefix}_rkT"][:])
                rvs = wp.tile([16, T, DH], BF16, tag="rvs")
                nc.sync.dma_start(out=rvs[:], in_=wts[f"{prefix}_rvs"][:])

                for half in range(2):
                    if filler is not None:
                        filler(half, qp)
                    wlo = half * HALFW
                    zT = ztp.tile([128, CHUNKS, HTOK], BF16, tag="zTt")
                    with ExitStack() as hs:
                        psz = hs.enter_context(
                            tc.tile_pool(name="pszT", bufs=4, space="PSUM"))
                        ln_fm(psz, zp, lambda w: x_sb[:, wlo + w, :], zT, HALFW)
                    qT = qp.tile([DH, HEADS, HTOK], BF16, tag="qt")
                    kT = qp.tile([DH, HEADS, HTOK], BF16, tag="kt")
                    with ExitStack() as hs:
                        psq = hs.enter_context(
                            tc.tile_pool(name="psqT", bufs=6, space="PSUM"))
                        proj_fm(psq, zT, wq, qT, HTOK)
                        proj_fm(psq, zT, wk, kT, HTOK)
                    v = qp.tile([128, HALFW, C], BF16, tag="vt")

                    def v_proj(psv):
                        # emitted after the window-loop prologue: the 12us of
                        # PE work fills the softmax pipeline-fill bubble (v is
                        # first consumed by CD(0)'s AV matmuls)
                        for w in range(HALFW):
                            pv = psv.tile([128, 1024], F32, tag="po2")
                            for o, n in ((0, 512), (512, 128)):
                                for ci in range(CHUNKS):
                                    nc.tensor.matmul(
                                        pv[:, o:o + n],
                                        zT[:, ci, 128 * w:128 * (w + 1)],
                                        wv[:, ci, o:o + n],
                                        start=(ci == 0), stop=(ci == CHUNKS - 1))
                            evict(v[:, w, :], pv[:, 0:C], w=(2, 1, 1))
                    # rel-pos shear: s2byT[r, t, h, j] = q_{r,t}.rk[j-t+16]
                    s2byT = sp2.tile([HR, T, HEADS, 16], BF16, tag="s2byT")
                    with ExitStack() as hs:
                        psh = hs.enter_context(
                            tc.tile_pool(name="pshT", bufs=3, space="PSUM"))
                        for h in range(HEADS):
                            pSB = zp.tile([NREL, HTOK], BF16, tag="pSB")
                            for o in range(0, HTOK, 512):
                                n = min(512, HTOK - o)
                                pp = psh.tile([NREL, 512], F32, tag="pp")
                                nc.tensor.matmul(pp[:, 0:n], rkT[:],
                                                 qT[:, h, o:o + n],
                                                 start=True, stop=True)
                                evict(pSB[:, o:o + n], pp[:, 0:n], w=(1, 1, 1))
                            pSa = pSB[:, :]
                            sh = psh.tile([HR, T, 64], BF16, tag="sh")
                            for t in range(T):
                                src = bass.AP(
                                    tensor=pSB.tensor, offset=pSa.offset + t,
                                    ap=[list(pSa.ap[0]), [16, HR]])
                                nc.tensor.transpose(sh[:, t, 0:NREL], src,
                                                    identb[:NREL, :NREL])
                            # sheared copy: col j of (r,t) = sh[r, t, 16-t+j]
                            sha = sh[:, :, :]
                            s2a = s2byT[:, :, :, :]
                            src = bass.AP(
                                tensor=sh.tensor, offset=sha.offset + 16,
                                ap=[list(sha.ap[0]), [63, 16], [1, 16]])
                            dst = bass.AP(
                                tensor=s2byT.tensor,
                                offset=s2a.offset + 16 * h,
                                ap=[list(s2a.ap[0]), [HEADS * 16, 16], [1, 16]])
                            evict(dst, src, w=(1, 1, 1))
                        # bounce via DRAM: s2_dram[(72*half+r)*16+t, h, j]
                        s2flat = s2_dram[:]
                        d_dst = bass.AP(tensor=s2flat.tensor,
                                        offset=s2flat.offset + half * HR * 2048,
                                        ap=[[2048, HR], [1, 2048]])
                        s2a = s2byT[:, :, :, :]
                        d_src = bass.AP(tensor=s2byT.tensor, offset=s2a.offset,
                                        ap=[list(s2a.ap[0]), [1, 2048]])
                        nc.sync.dma_start(out=d_dst, in_=d_src)

                    # per-window attention, 3-deep pipeline:
                    # fa(w+2) scores; fb(w+1) softmax+AV; back(w) WO+resid
                    with ExitStack() as hs:
                        psA = hs.enter_context(
                            tc.tile_pool(name="psAT", bufs=2, space="PSUM"))
                        psB = hs.enter_context(
                            tc.tile_pool(name="psBT", bufs=2, space="PSUM"))
                        psC = hs.enter_context(
                            tc.tile_pool(name="psCT", bufs=1, space="PSUM"))

                        def t_fa(w):
                            wg = wlo + w
                            s2w = swp.tile([128, HEADS, 16], BF16, tag="s2w")
                            nc.sync.dma_start(
                                out=s2w[:], in_=s2_dram[128 * wg:128 * (wg + 1)])
                            # emask = mask * exp(scale*s2w), built on Act/Pool
                            # off the critical path
                            eb = swp.tile([128, HEADS, 16], BF16, tag="eb")
                            nc.scalar.activation(out=eb[:], in_=s2w[:],
                                                 func=AF.Exp, scale=SCALE)
                            em = swp.tile([128, HEADS, 128], BF16, tag="em")
                            eba = eb[:, :, :]
                            ebr = bass.AP(tensor=eb.tensor, offset=eba.offset,
                                          ap=[list(eba.ap[0]), [16, HEADS],
                                              [0, 8], [1, 16]])
                            maska = mask[:, :]
                            maskr = bass.AP(tensor=mask.tensor,
                                            offset=maska.offset,
                                            ap=[list(maska.ap[0]), [0, HEADS],
                                                [1, 128]])
                            nc.gpsimd.tensor_tensor(out=em[:], in0=maskr,
                                                    in1=ebr, op=ALU.mult)
                            pS = psA.tile([128, 1024], F32, tag="pS")
                            for h in range(HEADS):
                                nc.tensor.matmul(
                                    pS[:, 128 * h:128 * (h + 1)],
                                    qT[:, h, 128 * w:128 * (w + 1)],
                                    kT[:, h, 128 * w:128 * (w + 1)],
                                    start=True, stop=True)
                            return pS, em

        # stage AB: exp + mask + softmax stats + normalize (Act/DVE/Pool)
                        def t_AB(w, pS, em):
                            aG = swp.tile([128, HEADS, 128], BF16, tag="aG")
                            nc.scalar.activation(out=aG[:], in_=pS[:],
                                                 func=AF.Exp, scale=SCALE)
                            nc.vector.tensor_tensor(out=aG[:], in0=aG[:],
                                                    in1=em[:], op=ALU.mult)
                            aD = swp.tile([128, HEADS, 16], F32, tag="aD")
                            aGa = aG[:, :, :]
                            agv = bass.AP(tensor=aG.tensor, offset=aGa.offset,
                                          ap=[list(aGa.ap[0]), [128, HEADS],
                                              [1, 16], [16, 8]])
                            nc.vector.tensor_reduce(out=aD[:], in_=agv,
                                                    axis=AX.X, op=ALU.add)
                            zt = small.tile([128, HEADS], F32, tag="zt")
                            nc.vector.tensor_reduce(out=zt[:], in_=aD[:],
                                                    axis=AX.X, op=ALU.add)
                            nc.vector.reciprocal(out=zt[:], in_=zt[:])
                            zta = zt[:, :]
                            rzb = bass.AP(tensor=zt.tensor, offset=zta.offset,
                                          ap=[list(zta.ap[0]), [1, HEADS],
                                              [0, 128]])
                            nc.gpsimd.tensor_tensor(out=aG[:], in0=aG[:],
                                                    in1=rzb, op=ALU.mult)
                            if debug and prefix == "t1" and wlo + w == 0:
                                nc.sync.dma_start(out=dbg["aG"][:], in_=aG[:])
                                nc.sync.dma_start(out=dbg["v0"][:],
                                                  in_=v[:, 0, :])
                            rzb2 = bass.AP(tensor=zt.tensor, offset=zta.offset,
                                           ap=[list(zta.ap[0]), [1, HEADS],
                                               [0, 16]])
                            aDn = swp.tile([128, HEADS, 16], BF16, tag="aDn")
                            nc.gpsimd.tensor_tensor(out=aDn[:], in0=aD[:],
                                                    in1=rzb2, op=ALU.mult)
                            return aG, aDn

                        # stage CD: transposes + AV (+rel-V) + oT assembly
                        def t_CD(w, aG, aDn):
                            paT = psB.tile([128, 1024], BF16, tag="ptr")
                            for h in range(HEADS):
                                nc.tensor.transpose(
                                    paT[:, 128 * h:128 * (h + 1)], aG[:, h, :],
                                    identb[:])
                            aTs = swp.tile([128, HEADS, 128], BF16, tag="aTs")
                            nc.scalar.copy(out=aTs[:], in_=paT[:])
                            pdT = psB.tile([128, 1024], BF16, tag="ptr")
                            for h in range(HEADS):
                                nc.tensor.transpose(
                                    pdT[:16, 128 * h:128 * (h + 1)],
                                    aDn[:, h, :], identb[:])
                            aDT = swp.tile([16, HEADS, 128], BF16, tag="aDT")
                            nc.scalar.copy(out=aDT[:], in_=pdT[:16, :])
                            # o1 = v^T A (plain start/stop groups per slot)
                            pO = psA.tile([128, 1024], F32, tag="pS")
                            for h in range(HEADS):
                                nc.tensor.matmul(pO[:DH, 128 * h:128 * (h + 1)],
                                                 v[:, w, DH * h:DH * (h + 1)],
                                                 aTs[:, h, :],
                                                 start=True, stop=True)
                            # o2: disjoint strided cols, own psum, no accum;
                            # one 64-col matmul per frame t (all heads+rows)
                            pR = psC.tile([128, 1024], F32, tag="po2")
                            aDa = aDT[:, :, :]
                            pRa = pR[:, :]
                            # one 32-col matmul per (frame t, head-half hh):
                            # the 4-head span stays inside one psum bank
                            for t in range(T):
                                for hh in range(2):
                                    off = 512 * hh + t
                                    rhs = bass.AP(
                                        tensor=aDT.tensor,
                                        offset=aDa.offset + off,
                                        ap=[list(aDa.ap[0]), [128, 4],
                                            [16, 8]])
                                    ov = bass.AP(
                                        tensor=pR.tensor,
                                        offset=pRa.offset + off,
                                        ap=[[pRa.ap[0][0], DH], [128, 4],
                                            [16, 8]])
                                    nc.tensor.matmul(ov, rvs[:, t, :], rhs,
                                                     start=True, stop=True)
                            oT = swp.tile([DH, HEADS, 128], BF16, tag="oTt")
                            pOa = pO[:, :]
                            src0 = bass.AP(tensor=pO.tensor, offset=pOa.offset,
                                           ap=[[pOa.ap[0][0], DH], [128, HEADS],
                                               [1, 128]])
                            src1 = bass.AP(tensor=pR.tensor, offset=pRa.offset,
                                           ap=[[pRa.ap[0][0], DH], [128, HEADS],
                                               [1, 128]])
                            nc.scalar.copy(out=oT[:, :, :], in_=src0)
                            nc.vector.tensor_tensor(out=oT[:, :, :], in0=src1,
                                                    in1=oT[:, :, :], op=ALU.add)
                            if debug and prefix == "t1" and wlo + w == 0:
                                nc.sync.dma_start(out=dbg["oT0"][:], in_=oT[:])
                            return oT

                        def t_back(w, oT):
                            wg = wlo + w
                            wo_resid(psA, "pS", oT, 0, 128, wo, x_sb[:, wg, :])
                            if debug:
                                nc.sync.dma_start(
                                    out=dbg[dbg_key][:].rearrange(
                                        "r t c -> (r t) c")[128 * wg:128 * (wg + 1), :],
                                    in_=x_sb[:, wg, :])

                        # staged pipeline, emission order chosen so every
                        # engine queue is in ready order (in-order queues):
                        # AB(k+1); CD(k); fa(k+2); back(k)
                        fa_q, ab_q = {}, {}
                        fa_q[0] = t_fa(0)
                        fa_q[1] = t_fa(1)
                        ab_q[0] = t_AB(0, *fa_q.pop(0))
                        v_proj(psC)
                        for k in range(HALFW):
                            if k + 1 < HALFW:
                                ab_q[k + 1] = t_AB(k + 1, *fa_q.pop(k + 1))
                            oT = t_CD(k, *ab_q.pop(k))
                            if k + 2 < HALFW:
                                fa_q[k + 2] = t_fa(k + 2)
                            t_back(k, oT)

        _mark("phaseA")
        temporal("t1", "t1", filler=fill_half)
        _mark("t1")

        # =====================================================================
        # Cross-attention
        # =====================================================================
        with ExitStack() as ph:
            zp = ph.enter_context(tc.tile_pool(name="zpX", bufs=2))
            qp = ph.enter_context(tc.tile_pool(name="qpX", bufs=2))

            for half in range(2):
                wlo = half * HALFW
                zT = zp.tile([128, CHUNKS, HTOK], BF16, tag="zTx")
                qT = qp.tile([DH, HEADS, HTOK], BF16, tag="qx")
                with ExitStack() as hs:
                    psz = hs.enter_context(tc.tile_pool(name="pszX", bufs=2,
                                                        space="PSUM"))
                    ln_fm(psz, zp, lambda w: x_sb[:, wlo + w, :], zT, HALFW)
                    proj_fm(psz, zT, wqx, qT, HTOK)
                with ExitStack() as hs:
                    pss = hs.enter_context(tc.tile_pool(name="pssX", bufs=2,
                                                        space="PSUM"))
                    psB = hs.enter_context(tc.tile_pool(name="psBX", bufs=2,
                                                        space="PSUM"))
                    eS = qp.tile([77, HEADS, HTOK], BF16, tag="eSx")

                    def escores(o):
                        # eS chunk [o, o+n): only needed by windows >= o//128,
                        # so later chunks are emitted after the window loop
                        # starts — their Act-bound exps overlap window PE work
                        n = min(512, HTOK - o)
                        for h in range(HEADS):
                            ps = pss.tile([77, 512], F32, tag="psx")
                            nc.tensor.matmul(ps[:, 0:n], kctxT[:, h, :],
                                             qT[:, h, o:o + n],
                                             start=True, stop=True)
                            nc.scalar.activation(out=eS[:, h, o:o + n],
                                                 in_=ps[:, 0:n],
                                                 func=AF.Exp, scale=SCALE)

                    def x_fa(w):
                        oX = psB.tile([128, 1024], F32, tag="oX")
                        for h in range(HEADS):
                            nc.tensor.matmul(oX[:, 128 * h:128 * h + DH + 1],
                                             eS[:, h, 128 * w:128 * (w + 1)],
                                             v1x[:, h, :],
                                             start=True, stop=True)
                        return oX

                    def x_fb(w, oX):
                        rz = small.tile([128, HEADS], F32, tag="rzx")
                        oXa = oX[:, :]
                        zv = bass.AP(tensor=oX.tensor, offset=oXa.offset + DH,
                                     ap=[list(oXa.ap[0]), [128, HEADS]])
                        nc.vector.reciprocal(out=rz[:], in_=zv)
                        oN = zp.tile([128, HEADS, DH], BF16, tag="oNx")
                        src = bass.AP(tensor=oX.tensor, offset=oXa.offset,
                                      ap=[list(oXa.ap[0]), [128, HEADS],
                                          [1, DH]])
                        rza = rz[:, :]
                        rzb = bass.AP(tensor=rz.tensor, offset=rza.offset,
                                      ap=[list(rza.ap[0]), [1, HEADS], [0, DH]])
                        nc.vector.tensor_tensor(out=oN[:], in0=src, in1=rzb,
                                                op=ALU.mult)
                        pt = psB.tile([DH, HEADS, 128], BF16, tag="ptx")
                        for h in range(HEADS):
                            nc.tensor.transpose(pt[:, h, :], oN[:, h, :],
                                                identb[:])
                        oTx = zp.tile([DH, HEADS, 128], BF16, tag="oTx")
                        evict(oTx[:], pt[:], w=(2, 1, 0))
                        return oTx

                    def x_back(w, oTx):
                        wg = wlo + w
                        wo_resid(psB, "oX", oTx, 0, 128, wox, x_sb[:, wg, :])
                        if debug:
                            nc.sync.dma_start(
                                out=dbg["x2"][:].rearrange(
                                    "r t c -> (r t) c")[128 * wg:128 * (wg + 1), :],
                                in_=x_sb[:, wg, :])

                    # staged: fb(k+1) before fa(k+2) before back(k), so each
                    # engine queue stays in ready order; eS score chunks for
                    # later windows are emitted mid-loop
                    escores(0)
                    fa_q, fb_q = {}, {}
                    fa_q[0] = x_fa(0)
                    fa_q[1] = x_fa(1)
                    fb_q[0] = x_fb(0, fa_q.pop(0))
                    escores(512)
                    for k in range(HALFW):
                        if k + 1 < HALFW:
                            fb_q[k + 1] = x_fb(k + 1, fa_q.pop(k + 1))
                        if k + 2 < HALFW:
                            fa_q[k + 2] = x_fa(k + 2)
                        x_back(k, fb_q.pop(k))
                        if k == 2:
                            escores(1024)

        _mark("cross")
        temporal("t2", "t2")
        _mark("t2")

        # =====================================================================
        # GEGLU FFN per window. ff_w1 cols host-permuted into rounds of
        # (4 a-chunks, 4 gate-chunks); a-chunk order preserved for ff_w2.
        # =====================================================================
        with ExitStack() as ph:
            wp = ph.enter_context(tc.tile_pool(name="wpF", bufs=1))
            zp = ph.enter_context(tc.tile_pool(name="zpF", bufs=2))
            hp = ph.enter_context(tc.tile_pool(name="hpF", bufs=2))
            psp = ph.enter_context(tc.tile_pool(name="psF", bufs=2, space="PSUM"))
            psx = ph.enter_context(tc.tile_pool(name="psxF", bufs=1, space="PSUM"))
            psh = ph.enter_context(tc.tile_pool(name="pshF", bufs=2, space="PSUM"))

            # w1 loaded per 1024-col round so the first super-window's
            # matmuls start after ~1/5 of the 6.5MB instead of all of it
            w1 = wp.tile([128, CHUNKS, 2 * FFI], BF16, tag="w1")
            for r in range(5):
                nc.sync.dma_start(
                    out=w1[:, :, 1024 * r:1024 * (r + 1)],
                    in_=wts["ff_w1"][:, 1024 * r:1024 * (r + 1)].rearrange(
                        "(a p) n -> p a n", p=128))
            w2 = wp.tile([128, NG2, C], BF16, tag="w2")
            for ci0 in range(0, NG2, 10):
                nc.sync.dma_start(
                    out=w2[:, ci0:ci0 + 10, :],
                    in_=wts["ff_w2"][128 * ci0:128 * (ci0 + 10), :].rearrange(
                        "(a p) n -> p a n", p=128))

            stats = ln_stats(wp, lambda w: x_sb[:, w, :], NWIN, "lnF")
            for sw in range(5):
                w0 = 4 * sw
                nw = 4 if sw < 4 else 2
                ntok = 128 * nw
                zT = zp.tile([128, CHUNKS, 512], BF16, tag="zTf")
                ln_fm(psp, zp, lambda i: x_sb[:, w0 + i, :], zT, nw,
                      stats=stats, w0=w0)
                uT = hp.tile([128, NG2, 512], BF16, tag="uT")
                for r in range(5):
                    for p in range(4):
                        ph_ = psh.tile([128, 2, 512], F32, tag="ph")
                        for j, co in ((0, p), (1, 4 + p)):
                            gcol = 1024 * r + 128 * co
                            for ci in range(CHUNKS):
                                nc.tensor.matmul(ph_[:, j, 0:ntok],
                                                 w1[:, ci, gcol:gcol + 128],
                                                 zT[:, ci, 0:ntok],
                                                 start=(ci == 0),
                                                 stop=(ci == CHUNKS - 1))
                        gl = hp.tile([128, 512], BF16, tag="gelu")
                        nc.scalar.activation(out=gl[:, 0:ntok],
                                             in_=ph_[:, 1, 0:ntok],
                                             func=AF.Gelu)
                        nc.vector.tensor_tensor(out=uT[:, 4 * r + p, 0:ntok],
                                                in0=ph_[:, 0, 0:ntok],
                                                in1=gl[:, 0:ntok],
                                                op=ALU.mult)
                for i in range(nw):
                    w = w0 + i
                    px = psx.tile([128, 1024], F32, tag="px")
                    for o, n in ((0, 512), (512, 128)):
                        for ci in range(NG2):
                            nc.tensor.matmul(px[:, o:o + n],
                                             uT[:, ci, 128 * i:128 * (i + 1)],
                                             w2[:, ci, o:o + n],
                                             start=(ci == 0), stop=(ci == NG2 - 1))
                    xout = zp.tile([128, C], F32, tag="xout")
                    nc.vector.scalar_tensor_tensor(
                        out=xout[:], in0=px[:, 0:C], scalar=1.0,
                        in1=x_sb[:, w, :], op0=ALU.mult, op1=ALU.add)
                    nc.sync.dma_start(
                        out=out_final[:].rearrange(
                            "r t c -> (r t) c")[128 * w:128 * (w + 1), :],
                        in_=xout[:])

    _mark("ffn")
    bass.BassTensorEngine.matmul = _omm
    if not nc.is_finalized():
        nc.finalize()
    return nc


# ----------------------------------------------------------------------------
# host side
# ----------------------------------------------------------------------------

def _bf(a):
    return np.asarray(a, dtype=ml_dtypes.bfloat16)


def prepare_inputs(inputs):
    f = {k: np.asarray(v, dtype=np.float32) for k, v in inputs.items()}
    shared = {}

    def fold(g, b, wname):
        wf = f[wname]
        bias = f[b] @ wf
        assert np.abs(bias).max() < 1e-6, f"nonzero folded bias for {wname}"
        return f[g][:, None] * wf

    for k in ("a1_bo", "a2_bo", "t1_bo", "t2_bo", "ff_b1", "ff_b2"):
        assert np.abs(f[k]).max() < 1e-6, f"nonzero bias {k} unsupported"

    for p, gk, bk_ in (("a1", "g1", "b1"), ("t1", "g4", "b4"),
                       ("t2", "g5", "b5")):
        for kind in ("wq", "wk", "wv"):
            shared[f"{p}_{kind}"] = _bf(fold(gk, bk_, f"{p}_{kind}"))
    shared["a2_wq"] = _bf(fold("g2", "b2", "a2_wq"))
    shared["a2_wk"] = _bf(f["a2_wk"])
    shared["a2_wv"] = _bf(f["a2_wv"])
    for p in ("a1", "a2", "t1", "t2"):
        shared[f"{p}_wo"] = _bf(
            f[f"{p}_wo"].reshape(HEADS, DH, C).transpose(1, 0, 2))
    for p in ("t1", "t2"):
        shared[f"{p}_rkT"] = _bf(f[f"{p}_rk"].T)
        rv = f[f"{p}_rv"]
        rvs = np.zeros((16, T, DH), np.float32)
        for t in range(T):
            for j in range(16):
                rvs[j, t] = rv[j - t + MAXREL]
        shared[f"{p}_rvs"] = _bf(rvs)
    w1f = fold("g3", "b3", "ff_w1")
    a_, g_ = w1f[:, :FFI], w1f[:, FFI:]
    cols = []
    for r in range(5):
        cols.append(a_[:, 512 * r:512 * (r + 1)])
        cols.append(g_[:, 512 * r:512 * (r + 1)])
    shared["ff_w1"] = _bf(np.concatenate(cols, axis=1))
    shared["ff_w2"] = _bf(f["ff_w2"])
    m = np.zeros((128, 128), np.float32)
    for g in range(8):
        m[16 * g:16 * (g + 1), 16 * g:16 * (g + 1)] = 1.0
    shared["bd_mask"] = _bf(m)

    x = f["x"]
    ctx = f["context"]
    in_maps = []
    for core in range(NCORES):
        im = dict(shared)
        xs = np.empty((NG, SEQ, C), np.float32)
        for g in range(NG):
            bt = core + 8 * g
            b, t = bt // T, bt % T
            xs[g] = x[b, :, t].reshape(C, SEQ).T
        im["xs_in"] = _bf(xs)
        im["ctxT"] = _bf(ctx[core // 4].T.copy())
        in_maps.append(im)
    return in_maps


_PROGRAM_CACHE = {}


def run(inputs, debug=False, trace=False):
    key = "dbg" if debug else "plain"
    if key not in _PROGRAM_CACHE:
        _PROGRAM_CACHE[key] = build_program(debug=debug)
    nc = _PROGRAM_CACHE[key]
    in_maps = prepare_inputs(inputs)
    from concourse.bass_utils import run_bass_kernel_spmd
    res = run_bass_kernel_spmd(nc, in_maps, list(range(NCORES)), trace=trace)
    outs = res.results
    full = np.empty((B * H * W, T, C), np.float32)
    for core in range(NCORES):
        full[NR * core:NR * (core + 1)] = outs[core]["out"]
    y = full.reshape(B, H, W, T, C).transpose(0, 4, 3, 1, 2)
    return y, res, outs


def kernel(**inputs):
    y, _, _ = run(inputs)
    return y.astype(np.float32)

